# revision 60
# baseline (speedup 1.0000x reference)
"""Collective-free causal attention: 8 cores = 4 batches x 2 q-stripe sets.

Core c = (batch b = c//2, stripe set h = c%2) owns the 8 query stripes
g = 2t + (1-h), t in 0..7, of batch b.  Each core projects the FULL K^T and V
for its batch locally (duplicated within the pair) instead of exchanging
halves over a collective — the cost model charges intra-pair AllGathers like
inter-chip transfers (15us + size/40GBps, serialized on one resource), which
dominated the original version; the duplicate K/V projection is ~27us of
extra PE work vs ~210us+ of modeled collective time.

Numerics: all matmul inputs are bf16 (host-converted); accumulation stays
fp32 in PSUM, softmax row sums stay fp32; outputs are stored bf16 and
widened to fp32 on the host.  Measured rel. Frobenius error ~5.4e-3.

Structure (single PE-dense stream, ~97% PE occupancy):
 - ~20 throwaway warm-up matmuls on a memset tile fill the input-DMA head
   so the cost model's PE clock ramp (half speed for the first 3us of a
   busy stretch) is spent before real work arrives.
 - Inputs land as a few large 3D-AP transfers (batched across the 8
   e-tiles), ordered so the K projection's first PSUM group is runnable
   ~5us in; W_k is host-shuffled to [k-chunk, p, e, c] so each k-chunk DMA
   moves 2KB rows and lands just ahead of the PE k-loop consuming it.
 - Projections K -> Q -> V (PSUM evictions alternate DVE/ACT); the
   single-block stripes t=0,1 compute scores/exp between Q and V and hold
   P^T + 1/rowsum in SBUF until the end.
 - Attention is stripe-major: each stripe walks its causal key blocks
   back-to-back, accumulating PV in one PSUM accumulation group (two
   stripes in flight across the vp pool's 2 tags x 3 bufs).  P^T is
   produced by an async SBUF->SBUF DMA transpose (XBAR) issued right after
   the exp, three items ahead of its PV, so its ~3us flight hides behind
   the scores stream — the PE never transposes.
 - Every stripe finalizes straight out of PSUM: the two feature halves are
   scaled by 1/rowsum on ACT and DVE in parallel (engine roles alternate
   per stripe) into a bf16 staging tile and stored with a single DMA.  The
   kernel tail is just PV -> scale -> store of a held 128-row stripe.
"""

import numpy as np

B, S, E, KD = 4, 2048, 1024, 1024
NCORES = 8
P = 128
ET = E // P          # 8 e-tiles of the contraction dim
KT = KD // P         # 8 k-tiles of Q^T/K^T partition dim
NQT = 8              # 8 q stripes of 128 per core
NBLK = 4             # 4 key blocks of 512
NST = S // P         # 16 key subtiles of 128 (V tiles)
NEG = -30000.0
SCALE = 1.0 / float(np.sqrt(KD))

_prog_cache = {}


def _n_blocks(t):
    return (t + 2) // 2


def _build_body(ctx, tc, ap):
    from concourse import mybir

    nc = tc.nc
    f32 = mybir.dt.float32
    bf16 = mybir.dt.bfloat16
    Exp = mybir.ActivationFunctionType.Exp
    X = mybir.AxisListType.X

    # batched [partition, e, cols] views of the inputs
    xTb = ap["xT"].rearrange("(e p) s -> p e s", p=P)      # [128, 8, 2048]
    xqb = ap["xTq"].rearrange("(e p) q -> p e q", p=P)     # [128, 8, 1024]
    wqb = ap["wqT"].rearrange("(e p) k -> p e k", p=P)
    # W_k is host-shuffled to [k-chunk, p, e, c] so each k-chunk DMA moves
    # 2KB-contiguous rows (256B rows would pay the sub-512B descriptor
    # penalty on the head-critical path)
    wkb = ap["wkS"].rearrange("k p e c -> k p (e c)")      # [8, 128, 1024]
    wvb = ap["wvT"].rearrange("(e p) f -> p e f", p=P)
    out_t = ap["out"].rearrange("(t p) f -> t p f", p=P)

    # ---- persistent tiles
    qt_pool = ctx.enter_context(tc.tile_pool(name="qt", bufs=1))
    QT = [qt_pool.tile([P, 1024], bf16, name=f"qt{k}", tag=f"qt{k}") for k in range(KT)]
    kt_pool = ctx.enter_context(tc.tile_pool(name="ktp", bufs=1))
    KTT = [kt_pool.tile([P, S], bf16, name=f"ktt{k}", tag=f"ktt{k}") for k in range(KT)]
    vv_pool = ctx.enter_context(tc.tile_pool(name="vvp", bufs=1))
    VV = [vv_pool.tile([P, E], bf16, name=f"vv{s}", tag=f"vv{s}") for s in range(NST)]
    rs_pool = ctx.enter_context(tc.tile_pool(name="rsp", bufs=1))
    RS = [rs_pool.tile([P, NBLK], f32, name=f"rs{t}", tag=f"rs{t}") for t in range(NQT)]
    const_pool = ctx.enter_context(tc.tile_pool(name="const", bufs=1))
    fin_pool = ctx.enter_context(tc.tile_pool(name="fin", bufs=4))

    # PSUM plan: sp lives for the whole kernel; pp (projection evictions,
    # 2 banks) is scoped to the projection phase and its banks are reused
    # by the attention vp pool (2 tags x 3 bufs = 6 banks; the handoff
    # dependency lands on the first PV matmuls, long after the last
    # projection eviction — no stall).
    sp = ctx.enter_context(tc.tile_pool(name="sp", bufs=2, space="PSUM"))

    # PE warm-up: the cost model runs the PE at 1/3.7 speed for the first
    # ~100ns of a busy stretch and at half speed until 3us of continuous
    # activity.  Fill the input-DMA head (~7us) with throwaway matmuls on a
    # memset tile so every real matmul runs at full rate.
    warm_sb = const_pool.tile([P, 256], bf16, name="warm_sb")
    nc.gpsimd.memset(warm_sb, 0)
    for i in range(22):
        wps = sp.tile([P, 256], f32, name="wps", tag="sp")
        nc.tensor.matmul(wps, warm_sb[:, :P], warm_sb, start=True, stop=True)

    p_pool = ctx.enter_context(tc.tile_pool(name="ppb", bufs=4))

    # GPSIMD cannot access PSUM, so evictions alternate DVE/ACT.
    evict_ops = [lambda d, s: nc.vector.tensor_copy(d, s),
                 lambda d, s: nc.scalar.copy(d, s)]
    evict_i = 0

    def evict(dst, src):
        nonlocal evict_i
        evict_ops[evict_i % 2](dst, src)
        evict_i += 1

    # ---- projection phase (scoped input pools + scoped eviction PSUM pool)
    held = {}  # t -> (P^T tiles, 1/rowsum) for the split stripes t=0,1
    with tc.tile_pool(name="xtp", bufs=1) as xt_pool, \
         tc.tile_pool(name="xqp", bufs=1) as xq_pool, \
         tc.tile_pool(name="wqp", bufs=1) as wq_pool, \
         tc.tile_pool(name="wkp", bufs=1) as wk_pool, \
         tc.tile_pool(name="wvp", bufs=1) as wv_pool, \
         tc.tile_pool(name="pp", bufs=2, space="PSUM") as pp:
        # x^T per key block sb: [p, (e 512)]  (slice e: [:, e*512:(e+1)*512])
        xts = [xt_pool.tile([P, ET * 512], bf16, name=f"xts{sb}", tag=f"xts{sb}")
               for sb in range(NBLK)]
        # W_k^T per k-chunk: [p, (e 128)]
        wkc = [wk_pool.tile([P, ET * P], bf16, name=f"wkc{k}", tag=f"wkc{k}")
               for k in range(KT)]
        wq_all = wq_pool.tile([P, ET * KD], bf16, name="wq", tag="wq")
        xq_all = xq_pool.tile([P, ET * 1024], bf16, name="xq", tag="xq")
        wv_all = wv_pool.tile([P, ET * E], bf16, name="wv", tag="wv")

        # DMA order tuned so the PE's first PSUM group is runnable ~5us in
        # and later chunks land just ahead of consumption.
        # first key block of x^T lands in two halves so the K projection's
        # e-accumulation can start after ~half the transfer
        xts0v = xts[0].rearrange("p (e s) -> p e s", s=512)
        nc.sync.dma_start(out=xts0v[:, 0:4, :], in_=xTb[:, 0:4, 0:512])
        nc.sync.dma_start(out=wkc[0], in_=wkb[0])
        nc.sync.dma_start(out=wkc[1], in_=wkb[1])
        nc.sync.dma_start(out=xts0v[:, 4:6, :], in_=xTb[:, 4:6, 0:512])
        nc.sync.dma_start(out=xts0v[:, 6:8, :], in_=xTb[:, 6:8, 0:512])
        for k in range(2, KT):
            nc.sync.dma_start(out=wkc[k], in_=wkb[k])
        for sb in range(1, NBLK):
            nc.sync.dma_start(
                out=xts[sb].rearrange("p (e s) -> p e s", s=512),
                in_=xTb[:, :, sb * 512:(sb + 1) * 512])
        nc.sync.dma_start(
            out=wq_all.rearrange("p (e k) -> p e k", k=KD), in_=wqb)
        nc.sync.dma_start(
            out=xq_all.rearrange("p (e q) -> p e q", q=1024), in_=xqb)
        nc.sync.dma_start(
            out=wv_all.rearrange("p (e f) -> p e f", f=E), in_=wvb)
        cm = const_pool.tile([P, 256], f32, name="cm")
        nc.sync.dma_start(out=cm, in_=ap["cmask"])

        # K^T[k] = W_k^T[:,k]^T x^T : [128 kdim, 2048 keys]
        for sb in range(NBLK):
            for k in range(KT):
                ps = pp.tile([P, 512], f32, name="ps_k", tag="pp")
                for e in range(ET):
                    nc.tensor.matmul(ps, wkc[k][:, e * P:(e + 1) * P],
                                     xts[sb][:, e * 512:(e + 1) * 512],
                                     start=(e == 0), stop=(e == ET - 1))
                evict(KTT[k][:, sb * 512:(sb + 1) * 512], ps)
        # Q^T[k] = W_q^T[:,k]^T xq^T : [128 kdim, 1024 own q rows]
        for qb in range(2):
            for k in range(KT):
                ps = pp.tile([P, 512], f32, name="ps_q", tag="pp")
                for e in range(ET):
                    nc.tensor.matmul(
                        ps, wq_all[:, e * KD + k * P: e * KD + (k + 1) * P],
                        xq_all[:, e * 1024 + qb * 512: e * 1024 + (qb + 1) * 512],
                        start=(e == 0), stop=(e == ET - 1))
                evict(QT[k][:, qb * 512:(qb + 1) * 512], ps)

        # scores/exp/transpose for the single-block stripes t=0,1 run here,
        # between the Q and V projections (their inputs — Q^T and key block
        # 0 — are ready); P^T and 1/rowsum are held in SBUF and only their
        # PV + store run at the very end of the kernel.
        for t in (1, 0):
            w = 256 * (t + 1)
            sps = sp.tile([P, 512], f32, name="sps", tag="sp")
            for k in range(KT):
                nc.tensor.matmul(sps[:, :w], QT[k][:, t * P:(t + 1) * P],
                                 KTT[k][:, :w], start=(k == 0),
                                 stop=(k == KT - 1))
            nc.vector.tensor_add(sps[:, w - 256:w], sps[:, w - 256:w], cm)
            pb = p_pool.tile([P, 512], bf16, name="pb", tag="pb")
            nc.scalar.activation(pb[:, :w], sps[:, :w], Exp, scale=SCALE,
                                 accum_out=RS[t][:, 0:1])
            ptile = fin_pool.tile([P, w], bf16, name="hpt", tag=f"hpt{t}",
                                  bufs=1)
            nc.sync.dma_start_transpose(
                out=ptile.rearrange("p (st c) -> p st c", c=P),
                in_=pb[:, :w])
            rinv = fin_pool.tile([P, 1], f32, name="hri", tag=f"hri{t}",
                                 bufs=1)
            nc.vector.reciprocal(rinv, RS[t][:, 0:1])
            held[t] = (ptile, w // P, rinv)

        # V[st] = x[st rows] W_v^T : [128 keys, 1024 features]
        for st in range(NST):
            sb, stv = st // 4, st % 4
            for fb in range(2):
                ps = pp.tile([P, 512], f32, name="ps_v", tag="pp")
                for e in range(ET):
                    nc.tensor.matmul(
                        ps, xts[sb][:, e * 512 + stv * P: e * 512 + (stv + 1) * P],
                        wv_all[:, e * E + fb * 512: e * E + (fb + 1) * 512],
                        start=(e == 0), stop=(e == ET - 1))
                evict(VV[st][:, fb * 512:(fb + 1) * 512], ps)

    # ---- attention phase
    vp = ctx.enter_context(tc.tile_pool(name="vp", bufs=2, space="PSUM"))
    pt_pool = ctx.enter_context(tc.tile_pool(name="ptp", bufs=5))

    # Attention is stripe-major: all of K^T/V is resident in SBUF, so each
    # stripe walks its key blocks back-to-back and accumulates PV entirely
    # in PSUM (vps holds both feature halves, one PSUM accumulation group
    # spanning the stripe's blocks).  No SBUF output accumulator, no DVE
    # adds, and every stripe finishes straight out of PSUM.  Only two
    # stripes' PV groups are ever in flight (vp tags x bufs=2 = 4 banks).
    cur_vps = {}
    fin_parity = [0]

    def scale_out(t, vps, rinv):
        obf = fin_pool.tile([P, E], bf16, name="obf", tag="obf", bufs=4)
        # scale the two halves on different engines in parallel (alternating
        # the assignment between consecutive finalizes so back-to-back
        # stripe finishes don't queue on one engine), then store with a
        # single DMA (HWDGE overhead is per-DMA)
        halves = [(0, vps[0]), (1, vps[1])]
        if fin_parity[0]:
            halves.reverse()
        fin_parity[0] ^= 1
        for i, (fb, src) in enumerate(halves):
            dst = obf[:, fb * 512:(fb + 1) * 512]
            if i == 0:
                nc.scalar.activation(dst, src,
                                     mybir.ActivationFunctionType.Copy,
                                     scale=rinv)
            else:
                nc.vector.tensor_scalar_mul(dst, src, rinv)
        nc.sync.dma_start(out=out_t[t], in_=obf)

    def finalize(t, vps):
        rsum = fin_pool.tile([P, 1], f32, name="rsum", tag="rsum")
        nc.vector.reduce_sum(rsum, RS[t][:, :_n_blocks(t)], axis=X)
        rinv = fin_pool.tile([P, 1], f32, name="rinv", tag="rinv")
        nc.vector.reciprocal(rinv, rsum)
        scale_out(t, vps, rinv)

    def emit_pv(pend):
        # deferred PV for one (t, blk) work item; P^T arrives via an async
        # DMA transpose issued right after the exp, two positions ahead, so
        # its ~3us flight time hides behind the scores stream.
        ptile, w, blk, t = pend
        nst = w // P
        if blk == 0:
            cur_vps[t] = [vp.tile([P, 512], f32, name=f"vps{fb}",
                                  tag=f"vp{fb}") for fb in range(2)]
        vps = cur_vps[t]
        is_final = (blk == _n_blocks(t) - 1)
        for st in range(nst):
            for fb in range(2):
                nc.tensor.matmul(vps[fb], ptile[:, st * P:(st + 1) * P],
                                 VV[4 * blk + st][:, fb * 512:(fb + 1) * 512],
                                 start=(blk == 0 and st == 0),
                                 stop=is_final and (st == nst - 1))
        if is_final:
            finalize(t, vps)

    # stripe-major schedule: big stripes first; the held single-block
    # stripes t=1,0 (scores pre-computed during the projection phase) come
    # last, so the kernel tail is just PV -> scale -> store.
    from collections import deque
    pend_q = deque()
    for t in (7, 3, 6, 2, 5, 4):
        for blk in range(_n_blocks(t)):
            w = min(512, 256 * (t + 1) - 512 * blk)
            is_diag = (blk == _n_blocks(t) - 1)
            sps = sp.tile([P, 512], f32, name="sps", tag="sp")
            for k in range(KT):
                nc.tensor.matmul(sps[:, :w], QT[k][:, t * P:(t + 1) * P],
                                 KTT[k][:, blk * 512: blk * 512 + w],
                                 start=(k == 0), stop=(k == KT - 1))
            if is_diag:
                nc.vector.tensor_add(sps[:, w - 256:w], sps[:, w - 256:w], cm)
            pb = p_pool.tile([P, 512], bf16, name="pb", tag="pb")
            nc.scalar.activation(pb[:, :w], sps[:, :w], Exp, scale=SCALE,
                                 accum_out=RS[t][:, blk:blk + 1])
            ptile = pt_pool.tile([P, 512], bf16, name="ptd", tag="ptd")
            nc.sync.dma_start_transpose(
                out=ptile.rearrange("p (st c) -> p st c", c=P)[:, :w // P, :],
                in_=pb[:, :w])
            pend_q.append((ptile, w, blk, t))
            if len(pend_q) > 3:
                emit_pv(pend_q.popleft())

    def emit_held_pv(t):
        # PV + PSUM-direct scale + store from pre-computed P^T and 1/rowsum
        ptile, nst, rinv = held[t]
        vps = [vp.tile([P, 512], f32, name=f"vps{fb}", tag=f"vp{fb}")
               for fb in range(2)]
        for st in range(nst):
            for fb in range(2):
                nc.tensor.matmul(vps[fb], ptile[:, st * P:(st + 1) * P],
                                 VV[st][:, fb * 512:(fb + 1) * 512],
                                 start=(st == 0), stop=(st == nst - 1))
        scale_out(t, vps, rinv)

    # drain: interleave the held stripes' PVs so each store chain hides
    # under the next stripe's PE work and the last in-flight DMA transpose
    # gets PE work to hide behind; only the very last store is exposed.
    emit_pv(pend_q.popleft())
    emit_pv(pend_q.popleft())
    emit_held_pv(1)
    emit_pv(pend_q.popleft())
    emit_held_pv(0)


def build_program():
    if "nc" in _prog_cache:
        return _prog_cache["nc"]
    from contextlib import ExitStack
    from concourse import bacc, mybir
    import concourse.tile as tile

    nc = bacc.Bacc("TRN2", target_bir_lowering=False, debug=False,
                   num_devices=NCORES)
    f32 = mybir.dt.float32
    bf16 = mybir.dt.bfloat16
    ap = {
        "xT": nc.dram_tensor("xT", [E, S], bf16, kind="ExternalInput").ap(),
        "xTq": nc.dram_tensor("xTq", [E, 1024], bf16, kind="ExternalInput").ap(),
        "wqT": nc.dram_tensor("wqT", [E, KD], bf16, kind="ExternalInput").ap(),
        "wkS": nc.dram_tensor("wkS", [KT, P, ET, P], bf16,
                              kind="ExternalInput").ap(),
        "wvT": nc.dram_tensor("wvT", [E, E], bf16, kind="ExternalInput").ap(),
        "cmask": nc.dram_tensor("cmask", [P, 256], f32, kind="ExternalInput").ap(),
        "out": nc.dram_tensor("out", [1024, E], bf16, kind="ExternalOutput").ap(),
    }
    with tile.TileContext(nc) as tc:
        with ExitStack() as ctx:
            _build_body(ctx, tc, ap)
    nc.compile()
    _prog_cache["nc"] = nc
    return nc


def make_in_maps(x, W_q, W_k, W_v):
    import ml_dtypes
    bf16 = ml_dtypes.bfloat16
    x = np.asarray(x, np.float32)
    wqT = np.ascontiguousarray(np.asarray(W_q, np.float32).T.astype(bf16))
    wkT = np.asarray(W_k, np.float32).T.astype(bf16)
    # [k-chunk, p, e, c]: wkS[k, p, e, c] = wkT[e*128+p, k*128+c]
    wkS = np.ascontiguousarray(
        wkT.reshape(ET, P, KT, P).transpose(2, 1, 0, 3))
    wvT = np.ascontiguousarray(np.asarray(W_v, np.float32).T.astype(bf16))
    i = np.arange(P)[:, None]
    j = np.arange(256)[None, :]
    cmasks = [np.where(j <= i + 128, 0.0, NEG).astype(np.float32),
              np.where(j <= i, 0.0, NEG).astype(np.float32)]
    in_maps = []
    for c in range(NCORES):
        b, h = c // 2, c % 2
        xT = x[b].T.astype(bf16)
        qtiles = [2 * t + (1 - h) for t in range(NQT)]
        qcols = np.concatenate([np.arange(g * P, (g + 1) * P) for g in qtiles])
        xTq = np.ascontiguousarray(xT[:, qcols])
        in_maps.append({
            "xT": np.ascontiguousarray(xT), "xTq": xTq,
            "wqT": wqT, "wkS": wkS, "wvT": wvT,
            "cmask": cmasks[h],
        })
    return in_maps


def assemble(results):
    out = np.zeros((B, S, E), np.float32)
    for c in range(NCORES):
        b, h = c // 2, c % 2
        co = np.asarray(results[c]["out"], dtype=np.float32)
        for t in range(NQT):
            g = 2 * t + (1 - h)
            out[b, g * P:(g + 1) * P, :] = co[t * P:(t + 1) * P]
    return out


def kernel(x, W_q, W_k, W_v):
    from concourse.bass_utils import run_bass_kernel_spmd
    nc = build_program()
    in_maps = make_in_maps(x, W_q, W_k, W_v)
    res = run_bass_kernel_spmd(nc, in_maps, core_ids=list(range(NCORES)))
    return assemble(res.results)


# revision 68
# speedup vs baseline: 1.1261x; 1.1261x over previous
"""Collective-free causal attention: 8 cores = 4 batches x 2 q-stripe sets.

Core c = (batch b = c//2, stripe set h = c%2) owns the 8 query stripes
g = 2t + (1-h), t in 0..7, of batch b.  Each core projects the FULL K^T and V
for its batch locally (duplicated within the pair) instead of exchanging
halves over a collective — the cost model charges intra-pair AllGathers like
inter-chip transfers (15us + size/40GBps, serialized on one resource), which
dominated the original version; the duplicate K/V projection is ~27us of
extra PE work vs ~210us+ of modeled collective time.

Numerics: all matmul inputs are bf16 (host-converted); accumulation stays
fp32 in PSUM, softmax row sums stay fp32; outputs are stored bf16 and
widened to fp32 on the host.  Measured rel. Frobenius error ~5.4e-3.

Structure (single PE-dense stream, ~97% PE occupancy):
 - ~20 throwaway warm-up matmuls on a memset tile fill the input-DMA head
   so the cost model's PE clock ramp (half speed for the first 3us of a
   busy stretch) is spent before real work arrives.
 - Inputs land as a few large 3D-AP transfers (batched across the 8
   e-tiles), ordered so the K projection's first PSUM group is runnable
   ~5us in; W_k is host-shuffled to [k-chunk, p, e, c] so each k-chunk DMA
   moves 2KB rows and lands just ahead of the PE k-loop consuming it.
 - Projections K -> Q -> V (PSUM evictions alternate DVE/ACT); the
   single-block stripes t=0,1 compute scores/exp between Q and V and hold
   P^T + 1/rowsum in SBUF until the end.
 - Attention is stripe-major: each stripe walks its causal key blocks
   back-to-back, accumulating PV in one PSUM accumulation group (two
   stripes in flight across the vp pool's 2 tags x 3 bufs).  P^T is
   produced by an async SBUF->SBUF DMA transpose (XBAR) issued right after
   the exp, three items ahead of its PV, so its ~3us flight hides behind
   the scores stream — the PE never transposes.
 - Every stripe finalizes straight out of PSUM: the two feature halves are
   scaled by 1/rowsum on ACT and DVE in parallel (engine roles alternate
   per stripe) into a bf16 staging tile and stored with a single DMA.  The
   kernel tail is just PV -> scale -> store of a held 128-row stripe.
"""

import numpy as np

B, S, E, KD = 4, 2048, 1024, 1024
NCORES = 8
P = 128
ET = E // P          # 8 e-tiles of the contraction dim
KT = KD // P         # 8 k-tiles of Q^T/K^T partition dim
NQT = 8              # 8 q stripes of 128 per core
NBLK = 4             # 4 key blocks of 512
NST = S // P         # 16 key subtiles of 128 (V tiles)
NEG = -30000.0
SCALE = 1.0 / float(np.sqrt(KD))

_prog_cache = {}


def _n_blocks(t):
    return (t + 2) // 2


def _build_body(ctx, tc, ap):
    from concourse import mybir

    nc = tc.nc
    f32 = mybir.dt.float32
    bf16 = mybir.dt.bfloat16
    Exp = mybir.ActivationFunctionType.Exp
    X = mybir.AxisListType.X

    # batched [partition, e, cols] views of the inputs
    xTb = ap["xT"].rearrange("(e p) s -> p e s", p=P)      # [128, 8, 2048]
    xqb = ap["xTq"].rearrange("(e p) q -> p e q", p=P)     # [128, 8, 1024]
    # W_q / W_k in their native [k, e] layout: k-tiles are the matmul
    # partition (contraction) dim of M = W_q^T W_k
    wqb = ap["wqP"].rearrange("(t p) e -> t p e", p=P)     # [8, 128, 1024]
    wkb = ap["wkP"].rearrange("(t p) f -> t p f", p=P)     # [8, 128, 1024]
    wvb = ap["wvT"].rearrange("(e p) f -> p e f", p=P)
    out_t = ap["out"].rearrange("(t p) f -> t p f", p=P)

    # ---- persistent tiles
    # YT[f-tile] = (x_q M)^T — plays the role Q^T played before: scores are
    # S = x_q M x^T with M = W_q^T W_k, so the full-sequence K projection
    # (the expensive duplicated half) is never materialized; scores contract
    # YT directly against the resident x^T key blocks.  Symmetrically the
    # full-sequence V projection is never materialized either: PV = (P x)
    # W_v^T, so the attention accumulates Z = P x against the resident
    # plain-x key tiles and multiplies by W_v^T once per 128-row stripe.
    qt_pool = ctx.enter_context(tc.tile_pool(name="qt", bufs=1))
    YT = [qt_pool.tile([P, 1024], bf16, name=f"yt{k}", tag=f"yt{k}") for k in range(KT)]
    # x^T per key block sb: [p, (e 512)] (slice e: [:, e*512:(e+1)*512])
    xt_pool = ctx.enter_context(tc.tile_pool(name="xtp", bufs=1))
    xts = [xt_pool.tile([P, ET * 512], bf16, name=f"xts{sb}", tag=f"xts{sb}")
           for sb in range(NBLK)]
    # plain x per key subtile: [128 keys, 1024 e] (Z rhs)
    xs_pool = ctx.enter_context(tc.tile_pool(name="xsp", bufs=1))
    XS = [xs_pool.tile([P, E], bf16, name=f"xs{s}", tag=f"xs{s}") for s in range(NST)]
    # W_v^T resident: [p, (e f)] (Z W_v^T rhs)
    wvp_pool = ctx.enter_context(tc.tile_pool(name="wvp", bufs=1))
    wv_all = wvp_pool.tile([P, ET * E], bf16, name="wv", tag="wv")
    rs_pool = ctx.enter_context(tc.tile_pool(name="rsp", bufs=1))
    RS = [rs_pool.tile([P, NBLK], f32, name=f"rs{t}", tag=f"rs{t}") for t in range(NQT)]
    const_pool = ctx.enter_context(tc.tile_pool(name="const", bufs=1))
    fin_pool = ctx.enter_context(tc.tile_pool(name="fin", bufs=4))

    # PSUM plan: sp lives for the whole kernel; pp (projection evictions,
    # 2 banks) is scoped to the projection phase and its banks are reused
    # by the attention vp pool (2 tags x 3 bufs = 6 banks; the handoff
    # dependency lands on the first PV matmuls, long after the last
    # projection eviction — no stall).
    sp = ctx.enter_context(tc.tile_pool(name="sp", bufs=2, space="PSUM"))

    # PE warm-up: the cost model runs the PE at 1/3.7 speed for the first
    # ~100ns of a busy stretch and at half speed until 3us of continuous
    # activity.  Fill the input-DMA head (~7us) with throwaway matmuls on a
    # memset tile so every real matmul runs at full rate.
    warm_sb = const_pool.tile([P, 256], bf16, name="warm_sb")
    nc.gpsimd.memset(warm_sb, 0)
    for i in range(56):
        wps = sp.tile([P, 256], f32, name="wps", tag="sp")
        nc.tensor.matmul(wps, warm_sb[:, :P], warm_sb, start=True, stop=True)

    p_pool = ctx.enter_context(tc.tile_pool(name="ppb", bufs=4))

    # GPSIMD cannot access PSUM, so evictions alternate DVE/ACT.
    evict_ops = [lambda d, s: nc.vector.tensor_copy(d, s),
                 lambda d, s: nc.scalar.copy(d, s)]
    evict_i = 0

    def evict(dst, src):
        nonlocal evict_i
        evict_ops[evict_i % 2](dst, src)
        evict_i += 1

    # ---- projection phase (scoped input pools + scoped eviction PSUM pool)
    held = {}  # t -> (P^T tiles, 1/rowsum) for the split stripes t=0,1
    with tc.tile_pool(name="xqp", bufs=1) as xq_pool, \
         tc.tile_pool(name="wqp", bufs=1) as wq_pool, \
         tc.tile_pool(name="wkp", bufs=1) as wk_pool, \
         tc.tile_pool(name="wvp", bufs=1) as wv_pool, \
         tc.tile_pool(name="mmp", bufs=1) as m_pool, \
         tc.tile_pool(name="pp", bufs=2, space="PSUM") as pp:
        wqt = [wq_pool.tile([P, E], bf16, name=f"wqt{k}", tag=f"wqt{k}")
               for k in range(KT)]
        wkt = [wk_pool.tile([P, E], bf16, name=f"wkt{k}", tag=f"wkt{k}")
               for k in range(KT)]
        MM = [m_pool.tile([P, E], bf16, name=f"mm{e}", tag=f"mm{e}")
              for e in range(ET)]
        xq_all = xq_pool.tile([P, ET * 1024], bf16, name="xq", tag="xq")
        wv_all = wv_pool.tile([P, ET * E], bf16, name="wv", tag="wv")
        wvv = wv_all.rearrange("p (e f) -> p e f", f=E)

        # DMA order tuned so the V projection's first PSUM group is
        # runnable ~8us in: x^T key block 0 (in pieces) + the fb0 half of
        # W_v^T arrive first; everything else lands with slack during V.
        xts0v = xts[0].rearrange("p (e s) -> p e s", s=512)
        nc.sync.dma_start(out=xts0v[:, 0:4, :], in_=xTb[:, 0:4, 0:512])
        nc.sync.dma_start(out=xts0v[:, 4:8, :], in_=xTb[:, 4:8, 0:512])
        nc.sync.dma_start(out=wvv[:, :, 0:512], in_=wvb[:, :, 0:512])
        nc.sync.dma_start(out=wvv[:, :, 512:1024], in_=wvb[:, :, 512:1024])
        for sb in range(1, NBLK):
            nc.sync.dma_start(
                out=xts[sb].rearrange("p (e s) -> p e s", s=512),
                in_=xTb[:, :, sb * 512:(sb + 1) * 512])
        for k in range(KT):
            nc.sync.dma_start(out=wqt[k], in_=wqb[k])
            nc.sync.dma_start(out=wkt[k], in_=wkb[k])
        nc.sync.dma_start(
            out=xq_all.rearrange("p (e q) -> p e q", q=1024), in_=xqb)
        cm = const_pool.tile([P, 256], f32, name="cm")
        nc.sync.dma_start(out=cm, in_=ap["cmask"])

        # V[st] = x[st rows] W_v^T : [128 keys, 1024 features]
        # (fb-outer within each key block so the second W_v half may land
        # while the first half's PSUM groups run)
        for sb in range(NBLK):
            for fb in range(2):
                for stv in range(4):
                    ps = pp.tile([P, 512], f32, name="ps_v", tag="pp")
                    for e in range(ET):
                        nc.tensor.matmul(
                            ps,
                            xts[sb][:, e * 512 + stv * P: e * 512 + (stv + 1) * P],
                            wv_all[:, e * E + fb * 512: e * E + (fb + 1) * 512],
                            start=(e == 0), stop=(e == ET - 1))
                    evict(VV[4 * sb + stv][:, fb * 512:(fb + 1) * 512], ps)

        # M[e,f] = sum_k W_q[k,e] W_k[k,f]  (the merged score operator)
        for et in range(ET):
            for fb in range(2):
                ps = pp.tile([P, 512], f32, name="ps_m", tag="pp")
                for kt in range(KT):
                    nc.tensor.matmul(ps, wqt[kt][:, et * P:(et + 1) * P],
                                     wkt[kt][:, fb * 512:(fb + 1) * 512],
                                     start=(kt == 0), stop=(kt == KT - 1))
                evict(MM[et][:, fb * 512:(fb + 1) * 512], ps)

        # YT[f-tile] = (x_q M)^T : [128 f, 1024 own q rows]
        for qb in range(2):
            for ft in range(KT):
                ps = pp.tile([P, 512], f32, name="ps_y", tag="pp")
                for et in range(ET):
                    nc.tensor.matmul(
                        ps, MM[et][:, ft * P:(ft + 1) * P],
                        xq_all[:, et * 1024 + qb * 512: et * 1024 + (qb + 1) * 512],
                        start=(et == 0), stop=(et == ET - 1))
                evict(YT[ft][:, qb * 512:(qb + 1) * 512], ps)

        # scores/exp/transpose for the single-block stripes t=0,1 run here
        # (their inputs — YT and x^T key block 0 — are ready); P^T and
        # 1/rowsum are held in SBUF and only their PV + store run at the
        # very end of the kernel.
        for t in (1, 0):
            w = 256 * (t + 1)
            sps = sp.tile([P, 512], f32, name="sps", tag="sp")
            for k in range(KT):
                nc.tensor.matmul(sps[:, :w], YT[k][:, t * P:(t + 1) * P],
                                 xts[0][:, k * 512: k * 512 + w],
                                 start=(k == 0), stop=(k == KT - 1))
            nc.vector.tensor_add(sps[:, w - 256:w], sps[:, w - 256:w], cm)
            pb = p_pool.tile([P, 512], bf16, name="pb", tag="pb")
            nc.scalar.activation(pb[:, :w], sps[:, :w], Exp, scale=SCALE,
                                 accum_out=RS[t][:, 0:1])
            ptile = fin_pool.tile([P, w], bf16, name="hpt", tag=f"hpt{t}",
                                  bufs=1)
            nc.sync.dma_start_transpose(
                out=ptile.rearrange("p (st c) -> p st c", c=P),
                in_=pb[:, :w])
            rinv = fin_pool.tile([P, 1], f32, name="hri", tag=f"hri{t}",
                                 bufs=1)
            nc.vector.reciprocal(rinv, RS[t][:, 0:1])
            held[t] = (ptile, w // P, rinv)

    # ---- attention phase
    vp = ctx.enter_context(tc.tile_pool(name="vp", bufs=2, space="PSUM"))
    pt_pool = ctx.enter_context(tc.tile_pool(name="ptp", bufs=5))

    # Attention is stripe-major: all of K^T/V is resident in SBUF, so each
    # stripe walks its key blocks back-to-back and accumulates PV entirely
    # in PSUM (vps holds both feature halves, one PSUM accumulation group
    # spanning the stripe's blocks).  No SBUF output accumulator, no DVE
    # adds, and every stripe finishes straight out of PSUM.  Only two
    # stripes' PV groups are ever in flight (vp tags x bufs=2 = 4 banks).
    cur_vps = {}
    fin_parity = [0]

    def scale_out(t, vps, rinv):
        obf = fin_pool.tile([P, E], bf16, name="obf", tag="obf", bufs=4)
        # scale the two halves on different engines in parallel (alternating
        # the assignment between consecutive finalizes so back-to-back
        # stripe finishes don't queue on one engine), then store with a
        # single DMA (HWDGE overhead is per-DMA)
        halves = [(0, vps[0]), (1, vps[1])]
        if fin_parity[0]:
            halves.reverse()
        fin_parity[0] ^= 1
        for i, (fb, src) in enumerate(halves):
            dst = obf[:, fb * 512:(fb + 1) * 512]
            if i == 0:
                nc.scalar.activation(dst, src,
                                     mybir.ActivationFunctionType.Copy,
                                     scale=rinv)
            else:
                nc.vector.tensor_scalar_mul(dst, src, rinv)
        nc.sync.dma_start(out=out_t[t], in_=obf)

    def finalize(t, vps):
        rsum = fin_pool.tile([P, 1], f32, name="rsum", tag="rsum")
        nc.vector.reduce_sum(rsum, RS[t][:, :_n_blocks(t)], axis=X)
        rinv = fin_pool.tile([P, 1], f32, name="rinv", tag="rinv")
        nc.vector.reciprocal(rinv, rsum)
        scale_out(t, vps, rinv)

    def emit_pv(pend):
        # deferred PV for one (t, blk) work item; P^T arrives via an async
        # DMA transpose issued right after the exp, two positions ahead, so
        # its ~3us flight time hides behind the scores stream.
        ptile, w, blk, t = pend
        nst = w // P
        if blk == 0:
            cur_vps[t] = [vp.tile([P, 512], f32, name=f"vps{fb}",
                                  tag=f"vp{fb}") for fb in range(2)]
        vps = cur_vps[t]
        is_final = (blk == _n_blocks(t) - 1)
        for st in range(nst):
            for fb in range(2):
                nc.tensor.matmul(vps[fb], ptile[:, st * P:(st + 1) * P],
                                 VV[4 * blk + st][:, fb * 512:(fb + 1) * 512],
                                 start=(blk == 0 and st == 0),
                                 stop=is_final and (st == nst - 1))
        if is_final:
            finalize(t, vps)

    # stripe-major schedule: big stripes first; the held single-block
    # stripes t=1,0 (scores pre-computed during the projection phase) come
    # last, so the kernel tail is just PV -> scale -> store.
    from collections import deque
    pend_q = deque()
    for t in (7, 3, 6, 2, 5, 4):
        for blk in range(_n_blocks(t)):
            w = min(512, 256 * (t + 1) - 512 * blk)
            is_diag = (blk == _n_blocks(t) - 1)
            sps = sp.tile([P, 512], f32, name="sps", tag="sp")
            for k in range(KT):
                nc.tensor.matmul(sps[:, :w], YT[k][:, t * P:(t + 1) * P],
                                 xts[blk][:, k * 512: k * 512 + w],
                                 start=(k == 0), stop=(k == KT - 1))
            if is_diag:
                nc.vector.tensor_add(sps[:, w - 256:w], sps[:, w - 256:w], cm)
            pb = p_pool.tile([P, 512], bf16, name="pb", tag="pb")
            nc.scalar.activation(pb[:, :w], sps[:, :w], Exp, scale=SCALE,
                                 accum_out=RS[t][:, blk:blk + 1])
            ptile = pt_pool.tile([P, 512], bf16, name="ptd", tag="ptd")
            nc.sync.dma_start_transpose(
                out=ptile.rearrange("p (st c) -> p st c", c=P)[:, :w // P, :],
                in_=pb[:, :w])
            pend_q.append((ptile, w, blk, t))
            if len(pend_q) > 3:
                emit_pv(pend_q.popleft())

    def emit_held_pv(t):
        # PV + PSUM-direct scale + store from pre-computed P^T and 1/rowsum
        ptile, nst, rinv = held[t]
        vps = [vp.tile([P, 512], f32, name=f"vps{fb}", tag=f"vp{fb}")
               for fb in range(2)]
        for st in range(nst):
            for fb in range(2):
                nc.tensor.matmul(vps[fb], ptile[:, st * P:(st + 1) * P],
                                 VV[st][:, fb * 512:(fb + 1) * 512],
                                 start=(st == 0), stop=(st == nst - 1))
        scale_out(t, vps, rinv)

    # drain: interleave the held stripes' PVs so each store chain hides
    # under the next stripe's PE work and the last in-flight DMA transpose
    # gets PE work to hide behind; only the very last store is exposed.
    emit_pv(pend_q.popleft())
    emit_pv(pend_q.popleft())
    emit_held_pv(1)
    emit_pv(pend_q.popleft())
    emit_held_pv(0)


def build_program():
    if "nc" in _prog_cache:
        return _prog_cache["nc"]
    from contextlib import ExitStack
    from concourse import bacc, mybir
    import concourse.tile as tile

    nc = bacc.Bacc("TRN2", target_bir_lowering=False, debug=False,
                   num_devices=NCORES)
    f32 = mybir.dt.float32
    bf16 = mybir.dt.bfloat16
    ap = {
        "xT": nc.dram_tensor("xT", [E, S], bf16, kind="ExternalInput").ap(),
        "xTq": nc.dram_tensor("xTq", [E, 1024], bf16, kind="ExternalInput").ap(),
        "wqP": nc.dram_tensor("wqP", [KD, E], bf16, kind="ExternalInput").ap(),
        "wkP": nc.dram_tensor("wkP", [KD, E], bf16, kind="ExternalInput").ap(),
        "wvT": nc.dram_tensor("wvT", [E, E], bf16, kind="ExternalInput").ap(),
        "cmask": nc.dram_tensor("cmask", [P, 256], f32, kind="ExternalInput").ap(),
        "out": nc.dram_tensor("out", [1024, E], bf16, kind="ExternalOutput").ap(),
    }
    with tile.TileContext(nc) as tc:
        with ExitStack() as ctx:
            _build_body(ctx, tc, ap)
    nc.compile()
    _prog_cache["nc"] = nc
    return nc


def make_in_maps(x, W_q, W_k, W_v):
    import ml_dtypes
    bf16 = ml_dtypes.bfloat16
    x = np.asarray(x, np.float32)
    wqP = np.ascontiguousarray(np.asarray(W_q, np.float32).astype(bf16))
    wkP = np.ascontiguousarray(np.asarray(W_k, np.float32).astype(bf16))
    wvT = np.ascontiguousarray(np.asarray(W_v, np.float32).T.astype(bf16))
    i = np.arange(P)[:, None]
    j = np.arange(256)[None, :]
    cmasks = [np.where(j <= i + 128, 0.0, NEG).astype(np.float32),
              np.where(j <= i, 0.0, NEG).astype(np.float32)]
    in_maps = []
    for c in range(NCORES):
        b, h = c // 2, c % 2
        xT = x[b].T.astype(bf16)
        qtiles = [2 * t + (1 - h) for t in range(NQT)]
        qcols = np.concatenate([np.arange(g * P, (g + 1) * P) for g in qtiles])
        xTq = np.ascontiguousarray(xT[:, qcols])
        in_maps.append({
            "xT": np.ascontiguousarray(xT), "xTq": xTq,
            "wqP": wqP, "wkP": wkP, "wvT": wvT,
            "cmask": cmasks[h],
        })
    return in_maps


def assemble(results):
    out = np.zeros((B, S, E), np.float32)
    for c in range(NCORES):
        b, h = c // 2, c % 2
        co = np.asarray(results[c]["out"], dtype=np.float32)
        for t in range(NQT):
            g = 2 * t + (1 - h)
            out[b, g * P:(g + 1) * P, :] = co[t * P:(t + 1) * P]
    return out


def kernel(x, W_q, W_k, W_v):
    from concourse.bass_utils import run_bass_kernel_spmd
    nc = build_program()
    in_maps = make_in_maps(x, W_q, W_k, W_v)
    res = run_bass_kernel_spmd(nc, in_maps, core_ids=list(range(NCORES)))
    return assemble(res.results)


# revision 79
# speedup vs baseline: 1.3036x; 1.1576x over previous
"""Collective-free causal attention: 8 cores = 4 batches x 2 q-stripe sets.

Core c = (batch b = c//2, stripe set h = c%2) owns the 8 query stripes
g = 2t + (1-h), t in 0..7, of batch b.  No collectives are used — the cost
model charges intra-pair AllGathers like inter-chip transfers (15us +
size/40GBps, serialized on one resource), which dominated the original
version.

The key restructure: the full-sequence K and V projections (the work that
would otherwise be duplicated within each core pair) are NEVER materialized.
With M = W_q^T W_k precomputed per core (E x E, cheap), scores are
S = (x_q M) x^T — the per-core q-side Y^T = (x_q M)^T contracts directly
against resident x^T key blocks.  Symmetrically PV = (P x) W_v^T: the
attention accumulates Z = P x against resident plain-x key tiles and
multiplies by W_v^T once per 128-row stripe.  Per-core PE work drops from
~198us (with duplicated K/V projections) to ~147us.

Numerics: all matmul inputs are bf16 (host-converted); accumulation stays
fp32 in PSUM, softmax row sums stay fp32; outputs are stored bf16 and
widened to fp32 on the host.  Measured rel. Frobenius error ~5.3e-3.

Overlap structure (single PE-dense stream):
 - ~56 throwaway warm-up matmuls on a memset tile fill the input-DMA head
   so the cost model's PE clock ramp (half speed for the first 3us of a
   busy stretch) is spent before real work arrives.
 - Inputs land as a few large batched transfers ordered so M's first PSUM
   group is runnable ~8.7us in and every later phase's operands land ahead
   of consumption.  Phases: M -> Y^T -> held scores (t=0,1) -> attention.
 - Attention is stripe-major: each stripe walks its causal key blocks
   back-to-back, accumulating Z in one PSUM accumulation group.  P^T is
   produced by an async SBUF->SBUF XBAR DMA transpose issued right after
   the exp, three items ahead of its Z matmuls — the PE never transposes.
   The stripe-final Z is evicted to bf16, XBAR-transposed (from the ACT
   DGE queue to dodge SP sequencer head-of-line blocking), multiplied by
   W_v^T three items later, scaled by 1/rowsum on ACT and DVE in parallel
   straight out of PSUM, and stored with a single DMA.
 - The single-block stripes t=0,1 precompute everything through Z^T early;
   the kernel tail is just their 16 Z W_v^T matmuls -> scale -> store.
"""

import numpy as np

B, S, E, KD = 4, 2048, 1024, 1024
NCORES = 8
P = 128
ET = E // P          # 8 e-tiles of the contraction dim
KT = KD // P         # 8 k-tiles of Q^T/K^T partition dim
NQT = 8              # 8 q stripes of 128 per core
NBLK = 4             # 4 key blocks of 512
NST = S // P         # 16 key subtiles of 128 (V tiles)
NEG = -30000.0
SCALE = 1.0 / float(np.sqrt(KD))

_prog_cache = {}


def _n_blocks(t):
    return (t + 2) // 2


def _build_body(ctx, tc, ap):
    from concourse import mybir

    nc = tc.nc
    f32 = mybir.dt.float32
    bf16 = mybir.dt.bfloat16
    Exp = mybir.ActivationFunctionType.Exp
    X = mybir.AxisListType.X

    # batched [partition, e, cols] views of the inputs
    xTb = ap["xT"].rearrange("(e p) s -> p e s", p=P)      # [128, 8, 2048]
    xqb = ap["xTq"].rearrange("(e p) q -> p e q", p=P)     # [128, 8, 1024]
    # W_q / W_k in their native [k, e] layout: k-tiles are the matmul
    # partition (contraction) dim of M = W_q^T W_k
    wqb = ap["wqP"].rearrange("(t p) e -> t p e", p=P)     # [8, 128, 1024]
    wkb = ap["wkP"].rearrange("(t p) f -> t p f", p=P)     # [8, 128, 1024]
    wvb = ap["wvT"].rearrange("(e p) f -> p e f", p=P)
    out_t = ap["out"].rearrange("(t p) f -> t p f", p=P)

    # ---- persistent tiles
    # YT[f-tile] = (x_q M)^T — plays the role Q^T played before: scores are
    # S = x_q M x^T with M = W_q^T W_k, so the full-sequence K projection
    # (the expensive duplicated half) is never materialized; scores contract
    # YT directly against the resident x^T key blocks.  Symmetrically the
    # full-sequence V projection is never materialized either: PV = (P x)
    # W_v^T, so the attention accumulates Z = P x against the resident
    # plain-x key tiles and multiplies by W_v^T once per 128-row stripe.
    qt_pool = ctx.enter_context(tc.tile_pool(name="qt", bufs=1))
    YT = [qt_pool.tile([P, 1024], bf16, name=f"yt{k}", tag=f"yt{k}") for k in range(KT)]
    # x^T per key block sb: [p, (e 512)] (slice e: [:, e*512:(e+1)*512])
    xt_pool = ctx.enter_context(tc.tile_pool(name="xtp", bufs=1))
    xts = [xt_pool.tile([P, ET * 512], bf16, name=f"xts{sb}", tag=f"xts{sb}")
           for sb in range(NBLK)]
    # plain x per key subtile: [128 keys, 1024 e] (Z rhs)
    xs_pool = ctx.enter_context(tc.tile_pool(name="xsp", bufs=1))
    XS = [xs_pool.tile([P, E], bf16, name=f"xs{s}", tag=f"xs{s}") for s in range(NST)]
    # W_v^T resident: [p, (e f)] (Z W_v^T rhs)
    wvp_pool = ctx.enter_context(tc.tile_pool(name="wvp", bufs=1))
    wv_all = wvp_pool.tile([P, ET * E], bf16, name="wv", tag="wv")
    rs_pool = ctx.enter_context(tc.tile_pool(name="rsp", bufs=1))
    RS = [rs_pool.tile([P, NBLK], f32, name=f"rs{t}", tag=f"rs{t}") for t in range(NQT)]
    const_pool = ctx.enter_context(tc.tile_pool(name="const", bufs=1))
    fin_pool = ctx.enter_context(tc.tile_pool(name="fin", bufs=4))

    # PSUM plan: sp lives for the whole kernel; pp (projection evictions,
    # 2 banks) is scoped to the projection phase and its banks are reused
    # by the attention vp pool (2 tags x 3 bufs = 6 banks; the handoff
    # dependency lands on the first PV matmuls, long after the last
    # projection eviction — no stall).
    sp = ctx.enter_context(tc.tile_pool(name="sp", bufs=2, space="PSUM"))

    # PE warm-up: the cost model runs the PE at 1/3.7 speed for the first
    # ~100ns of a busy stretch and at half speed until 3us of continuous
    # activity.  Fill the input-DMA head (~7us) with throwaway matmuls on a
    # memset tile so every real matmul runs at full rate.
    warm_sb = const_pool.tile([P, 256], bf16, name="warm_sb")
    nc.gpsimd.memset(warm_sb, 0)
    for i in range(56):
        wps = sp.tile([P, 256], f32, name="wps", tag="sp")
        nc.tensor.matmul(wps, warm_sb[:, :P], warm_sb, start=True, stop=True)

    p_pool = ctx.enter_context(tc.tile_pool(name="ppb", bufs=4))

    # GPSIMD cannot access PSUM, so evictions alternate DVE/ACT.
    evict_ops = [lambda d, s: nc.vector.tensor_copy(d, s),
                 lambda d, s: nc.scalar.copy(d, s)]
    evict_i = 0

    def evict(dst, src):
        nonlocal evict_i
        evict_ops[evict_i % 2](dst, src)
        evict_i += 1

    # ---- projection phase (scoped input pools + scoped eviction PSUM pool)
    held = {}  # t -> (P^T tiles, 1/rowsum) for the split stripes t=0,1
    with tc.tile_pool(name="xqp", bufs=1) as xq_pool, \
         tc.tile_pool(name="wqp", bufs=1) as wq_pool, \
         tc.tile_pool(name="wkp", bufs=1) as wk_pool, \
         tc.tile_pool(name="mmp", bufs=1) as m_pool, \
         tc.tile_pool(name="pp", bufs=2, space="PSUM") as pp:
        wq_all = wq_pool.tile([P, KT * E], bf16, name="wq", tag="wq")
        wk_all = wk_pool.tile([P, KT * E], bf16, name="wk", tag="wk")
        MM = [m_pool.tile([P, E], bf16, name=f"mm{e}", tag=f"mm{e}")
              for e in range(ET)]
        xq_all = xq_pool.tile([P, ET * 1024], bf16, name="xq", tag="xq")

        # DMA order: the M computation's first PSUM groups need the low
        # e-half of W_q plus the fb0 half of W_k (runnable ~8.7us in, in
        # four batched half-transfers); everything else lands with slack
        # during M/YT.
        wqv = wq_all.rearrange("p (k e) -> p k e", e=E)
        wkv = wk_all.rearrange("p (k f) -> p k f", f=E)
        wqb2 = ap["wqP"].rearrange("(k p) e -> p k e", p=P)
        wkb2 = ap["wkP"].rearrange("(k p) f -> p k f", p=P)
        nc.sync.dma_start(out=wqv[:, :, 0:512], in_=wqb2[:, :, 0:512])
        nc.sync.dma_start(out=wkv[:, :, 0:512], in_=wkb2[:, :, 0:512])
        nc.sync.dma_start(out=wqv[:, :, 512:1024], in_=wqb2[:, :, 512:1024])
        nc.sync.dma_start(out=wkv[:, :, 512:1024], in_=wkb2[:, :, 512:1024])
        nc.sync.dma_start(
            out=xq_all.rearrange("p (e q) -> p e q", q=1024), in_=xqb)
        for sb in range(NBLK):
            nc.sync.dma_start(
                out=xts[sb].rearrange("p (e s) -> p e s", s=512),
                in_=xTb[:, :, sb * 512:(sb + 1) * 512])
        xsb = ap["xP"].rearrange("(g p) e -> g p e", p=P)   # [16,128,1024]
        for g in range(NST):
            nc.sync.dma_start(out=XS[g], in_=xsb[g])
        nc.sync.dma_start(
            out=wv_all.rearrange("p (e f) -> p e f", f=E), in_=wvb)
        cm = const_pool.tile([P, 256], f32, name="cm")
        nc.sync.dma_start(out=cm, in_=ap["cmask"])

        # M[e,f] = sum_k W_q[k,e] W_k[k,f]  (the merged score operator);
        # fb-outer so the second W_k half may land while fb0 groups run
        for fb in range(2):
            for et in range(ET):
                ps = pp.tile([P, 512], f32, name="ps_m", tag="pp")
                for kt in range(KT):
                    nc.tensor.matmul(
                        ps, wq_all[:, kt * E + et * P: kt * E + (et + 1) * P],
                        wk_all[:, kt * E + fb * 512: kt * E + (fb + 1) * 512],
                        start=(kt == 0), stop=(kt == KT - 1))
                evict(MM[et][:, fb * 512:(fb + 1) * 512], ps)

        # YT[f-tile] = (x_q M)^T : [128 f, 1024 own q rows]
        for qb in range(2):
            for ft in range(KT):
                ps = pp.tile([P, 512], f32, name="ps_y", tag="pp")
                for et in range(ET):
                    nc.tensor.matmul(
                        ps, MM[et][:, ft * P:(ft + 1) * P],
                        xq_all[:, et * 1024 + qb * 512: et * 1024 + (qb + 1) * 512],
                        start=(et == 0), stop=(et == ET - 1))
                evict(YT[ft][:, qb * 512:(qb + 1) * 512], ps)

        # scores/exp/transpose for the single-block stripes t=0,1 run here
        # (their inputs — YT and x^T key block 0 — are ready); P^T and
        # 1/rowsum are held in SBUF and only their PV + store run at the
        # very end of the kernel.
        for t in (1, 0):
            w = 256 * (t + 1)
            sps = sp.tile([P, 512], f32, name="sps", tag="sp")
            for k in range(KT):
                nc.tensor.matmul(sps[:, :w], YT[k][:, t * P:(t + 1) * P],
                                 xts[0][:, k * 512: k * 512 + w],
                                 start=(k == 0), stop=(k == KT - 1))
            nc.vector.tensor_add(sps[:, w - 256:w], sps[:, w - 256:w], cm)
            pb = p_pool.tile([P, 512], bf16, name="pb", tag="pb")
            nc.scalar.activation(pb[:, :w], sps[:, :w], Exp, scale=SCALE,
                                 accum_out=RS[t][:, 0:1])
            ptile = fin_pool.tile([P, w], bf16, name="hpt", tag=f"hpt{t}",
                                  bufs=1)
            nc.sync.dma_start_transpose(
                out=ptile.rearrange("p (st c) -> p st c", c=P),
                in_=pb[:, :w])
            rinv = fin_pool.tile([P, 1], f32, name="hri", tag=f"hri{t}",
                                 bufs=1)
            nc.vector.reciprocal(rinv, RS[t][:, 0:1])
            held[t] = (ptile, w // P, rinv)

    # ---- attention phase
    vp = ctx.enter_context(tc.tile_pool(name="vp", bufs=2, space="PSUM"))
    op = ctx.enter_context(tc.tile_pool(name="op", bufs=1, space="PSUM"))
    pt_pool = ctx.enter_context(tc.tile_pool(name="ptp", bufs=5))

    # Attention is stripe-major: each stripe walks its causal key blocks
    # back-to-back, accumulating Z = P x in one PSUM accumulation group
    # (zps holds the two e-halves).  At the stripe's end Z is evicted to
    # bf16, transposed by an async XBAR DMA, multiplied by W_v^T (16
    # matmuls into the op PSUM pool) and scaled straight out of PSUM by
    # 1/rowsum.  The Z->out stage runs two work items behind its stripe so
    # the eviction + DMA-transpose flight hides behind the scores stream.
    cur_zps = {}
    fin_parity = [0]

    def scale_out(t, srcs, rinv):
        obf = fin_pool.tile([P, E], bf16, name="obf", tag="obf", bufs=4)
        # scale the two halves on different engines in parallel (alternating
        # the assignment between consecutive finalizes so back-to-back
        # stripe finishes don't queue on one engine), then store with a
        # single DMA (HWDGE overhead is per-DMA)
        halves = [(0, srcs[0]), (1, srcs[1])]
        if fin_parity[0]:
            halves.reverse()
        fin_parity[0] ^= 1
        for i, (fb, src) in enumerate(halves):
            dst = obf[:, fb * 512:(fb + 1) * 512]
            if i == 0:
                nc.scalar.activation(dst, src,
                                     mybir.ActivationFunctionType.Copy,
                                     scale=rinv)
            else:
                nc.vector.tensor_scalar_mul(dst, src, rinv)
        nc.sync.dma_start(out=out_t[t], in_=obf)

    def evict_z(t, zps, zt_tag, zt_bufs):
        # Z (PSUM) -> bf16 -> async XBAR transpose -> zt [e-in-tile,(et q)]
        zb = fin_pool.tile([P, E], bf16, name="zb", tag="zb", bufs=3)
        nc.vector.tensor_copy(zb[:, 0:512], zps[0])
        nc.scalar.copy(zb[:, 512:1024], zps[1])
        zt = fin_pool.tile([P, E], bf16, name="zt", tag=zt_tag, bufs=zt_bufs)
        # issued from the ACT engine DGE queue: the SP sequencer is busy with
        # P^T transposes and output stores (~590ns each, in-order), which
        # would head-of-line-block this latency-critical transpose
        nc.scalar.dma_start_transpose(
            out=zt.rearrange("p (et q) -> p et q", q=P), in_=zb)
        return zt

    def emit_zout(t, zt, rinv=None):
        # out = Z W_v^T accumulated in the op PSUM pool, scaled by 1/rowsum
        ops = [op.tile([P, 512], f32, name=f"ops{fb}", tag=f"op{fb}")
               for fb in range(2)]
        for et in range(ET):
            for fb in range(2):
                nc.tensor.matmul(ops[fb], zt[:, et * P:(et + 1) * P],
                                 wv_all[:, et * E + fb * 512: et * E + (fb + 1) * 512],
                                 start=(et == 0), stop=(et == ET - 1))
        if rinv is None:
            rsum = fin_pool.tile([P, 1], f32, name="rsum", tag="rsum")
            nc.vector.reduce_sum(rsum, RS[t][:, :_n_blocks(t)], axis=X)
            rinv = fin_pool.tile([P, 1], f32, name="rinv", tag="rinv")
            nc.vector.reciprocal(rinv, rsum)
        scale_out(t, ops, rinv)

    zout_q = []  # [(age, t, zt)]

    def tick_zout():
        for ent in list(zout_q):
            if ent[0] >= 3:
                zout_q.remove(ent)
                emit_zout(ent[1], ent[2])
            else:
                ent[0] += 1

    def emit_pv(pend):
        # deferred Z accumulation for one (t, blk) work item; P^T arrives
        # via an async DMA transpose issued right after the exp, three
        # positions ahead, so its flight time hides behind scores work.
        tick_zout()
        ptile, w, blk, t = pend
        nst = w // P
        if blk == 0:
            cur_zps[t] = [vp.tile([P, 512], f32, name=f"zps{fb}",
                                  tag=f"vp{fb}") for fb in range(2)]
        zps = cur_zps[t]
        is_final = (blk == _n_blocks(t) - 1)
        for st in range(nst):
            for eb in range(2):
                nc.tensor.matmul(zps[eb], ptile[:, st * P:(st + 1) * P],
                                 XS[4 * blk + st][:, eb * 512:(eb + 1) * 512],
                                 start=(blk == 0 and st == 0),
                                 stop=is_final and (st == nst - 1))
        if is_final:
            zt = evict_z(t, zps, "ztm", 2)
            zout_q.append([0, t, zt])

    # held stripes t=0,1: their Z is also fully precomputed here (P^T and
    # 1/rowsum were staged during the projection phase); only the 16
    # Z W_v^T matmuls + scale + store remain for the kernel tail.
    for t in (1, 0):
        ptile, nst, rinv = held[t]
        zps = [vp.tile([P, 512], f32, name=f"zps{fb}", tag=f"vp{fb}")
               for fb in range(2)]
        for st in range(nst):
            for eb in range(2):
                nc.tensor.matmul(zps[eb], ptile[:, st * P:(st + 1) * P],
                                 XS[st][:, eb * 512:(eb + 1) * 512],
                                 start=(st == 0), stop=(st == nst - 1))
        zt = evict_z(t, zps, f"hzt{t}", 1)
        held[t] = (zt, rinv)

    # stripe-major schedule: big stripes first; the held single-block
    # stripes t=1,0 come last, so the kernel tail is just the Z W_v^T
    # matmuls -> scale -> store.
    from collections import deque
    pend_q = deque()
    for t in (7, 3, 6, 2, 5, 4):
        for blk in range(_n_blocks(t)):
            w = min(512, 256 * (t + 1) - 512 * blk)
            is_diag = (blk == _n_blocks(t) - 1)
            sps = sp.tile([P, 512], f32, name="sps", tag="sp")
            for k in range(KT):
                nc.tensor.matmul(sps[:, :w], YT[k][:, t * P:(t + 1) * P],
                                 xts[blk][:, k * 512: k * 512 + w],
                                 start=(k == 0), stop=(k == KT - 1))
            if is_diag:
                nc.vector.tensor_add(sps[:, w - 256:w], sps[:, w - 256:w], cm)
            pb = p_pool.tile([P, 512], bf16, name="pb", tag="pb")
            nc.scalar.activation(pb[:, :w], sps[:, :w], Exp, scale=SCALE,
                                 accum_out=RS[t][:, blk:blk + 1])
            ptile = pt_pool.tile([P, 512], bf16, name="ptd", tag="ptd")
            nc.sync.dma_start_transpose(
                out=ptile.rearrange("p (st c) -> p st c", c=P)[:, :w // P, :],
                in_=pb[:, :w])
            pend_q.append((ptile, w, blk, t))
            if len(pend_q) > 3:
                emit_pv(pend_q.popleft())

    # drain: flush the remaining work items and Z->out stages, ending with
    # the held stripes so only the very last store chain is exposed.
    while pend_q:
        emit_pv(pend_q.popleft())
    tick_zout()
    while zout_q:
        ent = zout_q.pop(0)
        emit_zout(ent[1], ent[2])
    for t in (1, 0):
        zt, rinv = held[t]
        emit_zout(t, zt, rinv)


def build_program():
    if "nc" in _prog_cache:
        return _prog_cache["nc"]
    from contextlib import ExitStack
    from concourse import bacc, mybir
    import concourse.tile as tile

    nc = bacc.Bacc("TRN2", target_bir_lowering=False, debug=False,
                   num_devices=NCORES)
    f32 = mybir.dt.float32
    bf16 = mybir.dt.bfloat16
    ap = {
        "xT": nc.dram_tensor("xT", [E, S], bf16, kind="ExternalInput").ap(),
        "xP": nc.dram_tensor("xP", [S, E], bf16, kind="ExternalInput").ap(),
        "xTq": nc.dram_tensor("xTq", [E, 1024], bf16, kind="ExternalInput").ap(),
        "wqP": nc.dram_tensor("wqP", [KD, E], bf16, kind="ExternalInput").ap(),
        "wkP": nc.dram_tensor("wkP", [KD, E], bf16, kind="ExternalInput").ap(),
        "wvT": nc.dram_tensor("wvT", [E, E], bf16, kind="ExternalInput").ap(),
        "cmask": nc.dram_tensor("cmask", [P, 256], f32, kind="ExternalInput").ap(),
        "out": nc.dram_tensor("out", [1024, E], bf16, kind="ExternalOutput").ap(),
    }
    with tile.TileContext(nc) as tc:
        with ExitStack() as ctx:
            _build_body(ctx, tc, ap)
    nc.compile()
    _prog_cache["nc"] = nc
    return nc


def make_in_maps(x, W_q, W_k, W_v):
    import ml_dtypes
    bf16 = ml_dtypes.bfloat16
    x = np.asarray(x, np.float32)
    wqP = np.ascontiguousarray(np.asarray(W_q, np.float32).astype(bf16))
    wkP = np.ascontiguousarray(np.asarray(W_k, np.float32).astype(bf16))
    wvT = np.ascontiguousarray(np.asarray(W_v, np.float32).T.astype(bf16))
    i = np.arange(P)[:, None]
    j = np.arange(256)[None, :]
    cmasks = [np.where(j <= i + 128, 0.0, NEG).astype(np.float32),
              np.where(j <= i, 0.0, NEG).astype(np.float32)]
    in_maps = []
    for c in range(NCORES):
        b, h = c // 2, c % 2
        xb = x[b].astype(bf16)
        xT = xb.T
        qtiles = [2 * t + (1 - h) for t in range(NQT)]
        qcols = np.concatenate([np.arange(g * P, (g + 1) * P) for g in qtiles])
        xTq = np.ascontiguousarray(xT[:, qcols])
        in_maps.append({
            "xT": np.ascontiguousarray(xT), "xP": np.ascontiguousarray(xb),
            "xTq": xTq, "wqP": wqP, "wkP": wkP, "wvT": wvT,
            "cmask": cmasks[h],
        })
    return in_maps


def assemble(results):
    out = np.zeros((B, S, E), np.float32)
    for c in range(NCORES):
        b, h = c // 2, c % 2
        co = np.asarray(results[c]["out"], dtype=np.float32)
        for t in range(NQT):
            g = 2 * t + (1 - h)
            out[b, g * P:(g + 1) * P, :] = co[t * P:(t + 1) * P]
    return out


def kernel(x, W_q, W_k, W_v):
    from concourse.bass_utils import run_bass_kernel_spmd
    nc = build_program()
    in_maps = make_in_maps(x, W_q, W_k, W_v)
    res = run_bass_kernel_spmd(nc, in_maps, core_ids=list(range(NCORES)))
    return assemble(res.results)


# revision 87
# speedup vs baseline: 1.3213x; 1.0136x over previous
"""Collective-free causal attention: 8 cores = 4 batches x 2 q-stripe sets.

Core c = (batch b = c//2, stripe set h = c%2) owns the 8 query stripes
g = 2t + (1-h), t in 0..7, of batch b.  No collectives are used — the cost
model charges intra-pair AllGathers like inter-chip transfers (15us +
size/40GBps, serialized on one resource), which dominated the original
version.

The key restructure: the full-sequence K and V projections (the work that
would otherwise be duplicated within each core pair) are NEVER materialized.
With M = W_q^T W_k precomputed per core (E x E, cheap), scores are
S = (x_q M) x^T — the per-core q-side Y^T = (x_q M)^T contracts directly
against resident x^T key blocks.  Symmetrically PV = (P x) W_v^T: the
attention accumulates Z = P x against resident plain-x key tiles and
multiplies by W_v^T once per 128-row stripe.  Per-core PE work drops from
~198us (with duplicated K/V projections) to ~147us.

Numerics: all matmul inputs are bf16 (host-converted); accumulation stays
fp32 in PSUM, softmax row sums stay fp32; outputs are stored bf16 and
widened to fp32 on the host.  Measured rel. Frobenius error ~5.3e-3.

Overlap structure (single PE-dense stream):
 - ~56 throwaway warm-up matmuls on a memset tile fill the input-DMA head
   so the cost model's PE clock ramp (half speed for the first 3us of a
   busy stretch) is spent before real work arrives.
 - Inputs land as a few large batched transfers ordered so M's first PSUM
   group is runnable ~8.7us in and every later phase's operands land ahead
   of consumption.  Phases: M -> Y^T -> held scores (t=0,1) -> attention.
 - Attention is stripe-major: each stripe walks its causal key blocks
   back-to-back, accumulating Z in one PSUM accumulation group.  P^T is
   produced by an async SBUF->SBUF XBAR DMA transpose issued right after
   the exp, three items ahead of its Z matmuls — the PE never transposes.
   The stripe-final Z is evicted to bf16, XBAR-transposed (from the ACT
   DGE queue to dodge SP sequencer head-of-line blocking), multiplied by
   W_v^T three items later, scaled by 1/rowsum on ACT and DVE in parallel
   straight out of PSUM, and stored with a single DMA.
 - The single-block stripes t=0,1 precompute everything through Z^T early;
   the kernel tail is just their 16 Z W_v^T matmuls -> scale -> store.
"""

import numpy as np

B, S, E, KD = 4, 2048, 1024, 1024
NCORES = 8
P = 128
ET = E // P          # 8 e-tiles of the contraction dim
KT = KD // P         # 8 k-tiles of Q^T/K^T partition dim
NQT = 8              # 8 q stripes of 128 per core
NBLK = 4             # 4 key blocks of 512
NST = S // P         # 16 key subtiles of 128 (V tiles)
NEG = -30000.0
SCALE = 1.0 / float(np.sqrt(KD))

_prog_cache = {}


def _n_blocks(t):
    return (t + 2) // 2


def _build_body(ctx, tc, ap):
    from concourse import mybir

    nc = tc.nc
    f32 = mybir.dt.float32
    bf16 = mybir.dt.bfloat16
    Exp = mybir.ActivationFunctionType.Exp
    X = mybir.AxisListType.X

    # batched [partition, e, cols] views of the inputs
    xTb = ap["xT"].rearrange("(e p) s -> p e s", p=P)      # [128, 8, 2048]
    xqb = ap["xTq"].rearrange("(e p) q -> p e q", p=P)     # [128, 8, 1024]
    wvb = ap["wvT"].rearrange("(e p) f -> p e f", p=P)
    out_t = ap["out"].rearrange("(t p) f -> t p f", p=P)

    # ---- persistent tiles
    # YT[f-tile] = (x_q M)^T — plays the role Q^T played before: scores are
    # S = x_q M x^T with M = W_q^T W_k, so the full-sequence K projection
    # (the expensive duplicated half) is never materialized; scores contract
    # YT directly against the resident x^T key blocks.  Symmetrically the
    # full-sequence V projection is never materialized either: PV = (P x)
    # W_v^T, so the attention accumulates Z = P x against the resident
    # plain-x key tiles and multiplies by W_v^T once per 128-row stripe.
    qt_pool = ctx.enter_context(tc.tile_pool(name="qt", bufs=1))
    YT = [qt_pool.tile([P, 1024], bf16, name=f"yt{k}", tag=f"yt{k}") for k in range(KT)]
    # x^T per key block sb: [p, (e 512)] (slice e: [:, e*512:(e+1)*512])
    xt_pool = ctx.enter_context(tc.tile_pool(name="xtp", bufs=1))
    xts = [xt_pool.tile([P, ET * 512], bf16, name=f"xts{sb}", tag=f"xts{sb}")
           for sb in range(NBLK)]
    # plain x per key subtile: [128 keys, 1024 e] (Z rhs)
    xs_pool = ctx.enter_context(tc.tile_pool(name="xsp", bufs=1))
    XS = [xs_pool.tile([P, E], bf16, name=f"xs{s}", tag=f"xs{s}") for s in range(NST)]
    # W_v^T resident: [p, (e f)] (Z W_v^T rhs)
    wvp_pool = ctx.enter_context(tc.tile_pool(name="wvp", bufs=1))
    wv_all = wvp_pool.tile([P, ET * E], bf16, name="wv", tag="wv")
    rs_pool = ctx.enter_context(tc.tile_pool(name="rsp", bufs=1))
    RS = [rs_pool.tile([P, NBLK], f32, name=f"rs{t}", tag=f"rs{t}") for t in range(NQT)]
    const_pool = ctx.enter_context(tc.tile_pool(name="const", bufs=1))
    fin_pool = ctx.enter_context(tc.tile_pool(name="fin", bufs=4))

    # PSUM plan: sp lives for the whole kernel; pp (projection evictions,
    # 2 banks) is scoped to the projection phase and its banks are reused
    # by the attention vp pool (2 tags x 3 bufs = 6 banks; the handoff
    # dependency lands on the first PV matmuls, long after the last
    # projection eviction — no stall).
    sp = ctx.enter_context(tc.tile_pool(name="sp", bufs=2, space="PSUM"))

    # PE warm-up: the cost model runs the PE at 1/3.7 speed for the first
    # ~100ns of a busy stretch and at half speed until 3us of continuous
    # activity.  Fill the input-DMA head (~7us) with throwaway matmuls on a
    # memset tile so every real matmul runs at full rate.
    warm_sb = const_pool.tile([P, 256], bf16, name="warm_sb")
    nc.gpsimd.memset(warm_sb, 0)
    for i in range(37):
        wps = sp.tile([P, 256], f32, name="wps", tag="sp")
        nc.tensor.matmul(wps, warm_sb[:, :P], warm_sb, start=True, stop=True)

    p_pool = ctx.enter_context(tc.tile_pool(name="ppb", bufs=4))

    # GPSIMD cannot access PSUM, so evictions alternate DVE/ACT.
    evict_ops = [lambda d, s: nc.vector.tensor_copy(d, s),
                 lambda d, s: nc.scalar.copy(d, s)]
    evict_i = 0

    def evict(dst, src):
        nonlocal evict_i
        evict_ops[evict_i % 2](dst, src)
        evict_i += 1

    # ---- projection phase (scoped input pools + scoped eviction PSUM pool)
    held = {}  # t -> (P^T tiles, 1/rowsum) for the split stripes t=0,1
    with tc.tile_pool(name="xqp", bufs=1) as xq_pool, \
         tc.tile_pool(name="wqp", bufs=1) as wq_pool, \
         tc.tile_pool(name="wkp", bufs=1) as wk_pool, \
         tc.tile_pool(name="mmp", bufs=1) as m_pool, \
         tc.tile_pool(name="pp", bufs=2, space="PSUM") as pp:
        # W_q host-shuffled to [e-chunk, p, k-tile, c]: each e-chunk DMA
        # moves 2KB-contiguous rows and feeds the M e-loop just in time
        wqc = [wq_pool.tile([P, KT * P], bf16, name=f"wqc{e}", tag=f"wqc{e}")
               for e in range(ET)]
        wk_all = wk_pool.tile([P, KT * E], bf16, name="wk", tag="wk")
        MM = [m_pool.tile([P, E], bf16, name=f"mm{e}", tag=f"mm{e}")
              for e in range(ET)]
        xq_all = xq_pool.tile([P, ET * 1024], bf16, name="xq", tag="xq")

        # DMA order: the M computation's first PSUM group needs the e0
        # chunk of W_q plus the fb0 half of W_k (runnable ~6.5us in); the
        # remaining W_q chunks and the W_k fb1 half land just ahead of the
        # PE's M loop consuming them.
        wkv = wk_all.rearrange("p (k f) -> p k f", f=E)
        wqS = ap["wqS"].rearrange("e p k c -> e p (k c)")   # [8, 128, 1024]
        wkb2 = ap["wkP"].rearrange("(k p) f -> p k f", p=P)
        nc.sync.dma_start(out=wqc[0], in_=wqS[0])
        nc.sync.dma_start(out=wkv[:, :, 0:512], in_=wkb2[:, :, 0:512])
        for e in range(1, ET):
            nc.sync.dma_start(out=wqc[e], in_=wqS[e])
        nc.sync.dma_start(out=wkv[:, :, 512:1024], in_=wkb2[:, :, 512:1024])
        nc.sync.dma_start(
            out=xq_all.rearrange("p (e q) -> p e q", q=1024), in_=xqb)
        for sb in range(NBLK):
            nc.sync.dma_start(
                out=xts[sb].rearrange("p (e s) -> p e s", s=512),
                in_=xTb[:, :, sb * 512:(sb + 1) * 512])
        xsb = ap["xP"].rearrange("(g p) e -> g p e", p=P)   # [16,128,1024]
        for g in range(NST):
            nc.sync.dma_start(out=XS[g], in_=xsb[g])
        nc.sync.dma_start(
            out=wv_all.rearrange("p (e f) -> p e f", f=E), in_=wvb)
        cm = const_pool.tile([P, 256], f32, name="cm")
        nc.sync.dma_start(out=cm, in_=ap["cmask"])

        # M[e,f] = sum_k W_q[k,e] W_k[k,f]  (the merged score operator);
        # fb-outer so the second W_k half may land while fb0 groups run
        for fb in range(2):
            for et in range(ET):
                ps = pp.tile([P, 512], f32, name="ps_m", tag="pp")
                for kt in range(KT):
                    nc.tensor.matmul(
                        ps, wqc[et][:, kt * P:(kt + 1) * P],
                        wk_all[:, kt * E + fb * 512: kt * E + (fb + 1) * 512],
                        start=(kt == 0), stop=(kt == KT - 1))
                evict(MM[et][:, fb * 512:(fb + 1) * 512], ps)

        # YT[f-tile] = (x_q M)^T : [128 f, 1024 own q rows]
        for qb in range(2):
            for ft in range(KT):
                ps = pp.tile([P, 512], f32, name="ps_y", tag="pp")
                for et in range(ET):
                    nc.tensor.matmul(
                        ps, MM[et][:, ft * P:(ft + 1) * P],
                        xq_all[:, et * 1024 + qb * 512: et * 1024 + (qb + 1) * 512],
                        start=(et == 0), stop=(et == ET - 1))
                evict(YT[ft][:, qb * 512:(qb + 1) * 512], ps)

        # scores/exp/transpose for the single-block stripes t=0,1 run here
        # (their inputs — YT and x^T key block 0 — are ready); P^T and
        # 1/rowsum are held in SBUF and only their PV + store run at the
        # very end of the kernel.
        for t in (1, 0):
            w = 256 * (t + 1)
            sps = sp.tile([P, 512], f32, name="sps", tag="sp")
            for k in range(KT):
                nc.tensor.matmul(sps[:, :w], YT[k][:, t * P:(t + 1) * P],
                                 xts[0][:, k * 512: k * 512 + w],
                                 start=(k == 0), stop=(k == KT - 1))
            nc.vector.tensor_add(sps[:, w - 256:w], sps[:, w - 256:w], cm)
            pb = p_pool.tile([P, 512], bf16, name="pb", tag="pb")
            nc.scalar.activation(pb[:, :w], sps[:, :w], Exp, scale=SCALE,
                                 accum_out=RS[t][:, 0:1])
            ptile = fin_pool.tile([P, w], bf16, name="hpt", tag=f"hpt{t}",
                                  bufs=1)
            nc.sync.dma_start_transpose(
                out=ptile.rearrange("p (st c) -> p st c", c=P),
                in_=pb[:, :w])
            rinv = fin_pool.tile([P, 1], f32, name="hri", tag=f"hri{t}",
                                 bufs=1)
            nc.vector.reciprocal(rinv, RS[t][:, 0:1])
            held[t] = (ptile, w // P, rinv)

    # ---- attention phase
    vp = ctx.enter_context(tc.tile_pool(name="vp", bufs=2, space="PSUM"))
    op = ctx.enter_context(tc.tile_pool(name="op", bufs=1, space="PSUM"))
    pt_pool = ctx.enter_context(tc.tile_pool(name="ptp", bufs=5))

    # Attention is stripe-major: each stripe walks its causal key blocks
    # back-to-back, accumulating Z = P x in one PSUM accumulation group
    # (zps holds the two e-halves).  At the stripe's end Z is evicted to
    # bf16, transposed by an async XBAR DMA, multiplied by W_v^T (16
    # matmuls into the op PSUM pool) and scaled straight out of PSUM by
    # 1/rowsum.  The Z->out stage runs two work items behind its stripe so
    # the eviction + DMA-transpose flight hides behind the scores stream.
    cur_zps = {}
    fin_parity = [0]

    def scale_out(t, srcs, rinv):
        obf = fin_pool.tile([P, E], bf16, name="obf", tag="obf", bufs=4)
        # scale the two halves on different engines in parallel (alternating
        # the assignment between consecutive finalizes so back-to-back
        # stripe finishes don't queue on one engine), then store with a
        # single DMA (HWDGE overhead is per-DMA)
        halves = [(0, srcs[0]), (1, srcs[1])]
        if fin_parity[0]:
            halves.reverse()
        fin_parity[0] ^= 1
        for i, (fb, src) in enumerate(halves):
            dst = obf[:, fb * 512:(fb + 1) * 512]
            if i == 0:
                nc.scalar.activation(dst, src,
                                     mybir.ActivationFunctionType.Copy,
                                     scale=rinv)
            else:
                nc.vector.tensor_scalar_mul(dst, src, rinv)
        nc.sync.dma_start(out=out_t[t], in_=obf)

    def evict_z(t, zps, zt_tag, zt_bufs):
        # Z (PSUM) -> bf16 -> async XBAR transpose -> zt [e-in-tile,(et q)]
        zb = fin_pool.tile([P, E], bf16, name="zb", tag="zb", bufs=3)
        nc.vector.tensor_copy(zb[:, 0:512], zps[0])
        nc.scalar.copy(zb[:, 512:1024], zps[1])
        zt = fin_pool.tile([P, E], bf16, name="zt", tag=zt_tag, bufs=zt_bufs)
        # issued from the ACT engine DGE queue: the SP sequencer is busy with
        # P^T transposes and output stores (~590ns each, in-order), which
        # would head-of-line-block this latency-critical transpose
        nc.scalar.dma_start_transpose(
            out=zt.rearrange("p (et q) -> p et q", q=P), in_=zb)
        return zt

    def emit_zout(t, zt, rinv=None):
        # out = Z W_v^T accumulated in the op PSUM pool, scaled by 1/rowsum
        ops = [op.tile([P, 512], f32, name=f"ops{fb}", tag=f"op{fb}")
               for fb in range(2)]
        for et in range(ET):
            for fb in range(2):
                nc.tensor.matmul(ops[fb], zt[:, et * P:(et + 1) * P],
                                 wv_all[:, et * E + fb * 512: et * E + (fb + 1) * 512],
                                 start=(et == 0), stop=(et == ET - 1))
        if rinv is None:
            rsum = fin_pool.tile([P, 1], f32, name="rsum", tag="rsum")
            nc.vector.reduce_sum(rsum, RS[t][:, :_n_blocks(t)], axis=X)
            rinv = fin_pool.tile([P, 1], f32, name="rinv", tag="rinv")
            nc.vector.reciprocal(rinv, rsum)
        scale_out(t, ops, rinv)

    zout_q = []  # [(age, t, zt)]

    def tick_zout():
        for ent in list(zout_q):
            if ent[0] >= 3:
                zout_q.remove(ent)
                emit_zout(ent[1], ent[2])
            else:
                ent[0] += 1

    def emit_pv(pend):
        # deferred Z accumulation for one (t, blk) work item; P^T arrives
        # via an async DMA transpose issued right after the exp, three
        # positions ahead, so its flight time hides behind scores work.
        tick_zout()
        ptile, w, blk, t = pend
        nst = w // P
        if blk == 0:
            cur_zps[t] = [vp.tile([P, 512], f32, name=f"zps{fb}",
                                  tag=f"vp{fb}") for fb in range(2)]
        zps = cur_zps[t]
        is_final = (blk == _n_blocks(t) - 1)
        for st in range(nst):
            for eb in range(2):
                nc.tensor.matmul(zps[eb], ptile[:, st * P:(st + 1) * P],
                                 XS[4 * blk + st][:, eb * 512:(eb + 1) * 512],
                                 start=(blk == 0 and st == 0),
                                 stop=is_final and (st == nst - 1))
        if is_final:
            zt = evict_z(t, zps, "ztm", 2)
            zout_q.append([0, t, zt])

    # held stripes t=0,1: their Z is also fully precomputed here (P^T and
    # 1/rowsum were staged during the projection phase); only the 16
    # Z W_v^T matmuls + scale + store remain for the kernel tail.
    for t in (1, 0):
        ptile, nst, rinv = held[t]
        zps = [vp.tile([P, 512], f32, name=f"zps{fb}", tag=f"vp{fb}")
               for fb in range(2)]
        for st in range(nst):
            for eb in range(2):
                nc.tensor.matmul(zps[eb], ptile[:, st * P:(st + 1) * P],
                                 XS[st][:, eb * 512:(eb + 1) * 512],
                                 start=(st == 0), stop=(st == nst - 1))
        zt = evict_z(t, zps, f"hzt{t}", 1)
        held[t] = (zt, rinv)

    # stripe-major schedule: big stripes first; the held single-block
    # stripes t=1,0 come last, so the kernel tail is just the Z W_v^T
    # matmuls -> scale -> store.
    from collections import deque
    pend_q = deque()
    for t in (7, 3, 6, 2, 5, 4):
        for blk in range(_n_blocks(t)):
            w = min(512, 256 * (t + 1) - 512 * blk)
            is_diag = (blk == _n_blocks(t) - 1)
            sps = sp.tile([P, 512], f32, name="sps", tag="sp")
            for k in range(KT):
                nc.tensor.matmul(sps[:, :w], YT[k][:, t * P:(t + 1) * P],
                                 xts[blk][:, k * 512: k * 512 + w],
                                 start=(k == 0), stop=(k == KT - 1))
            if is_diag:
                nc.vector.tensor_add(sps[:, w - 256:w], sps[:, w - 256:w], cm)
            pb = p_pool.tile([P, 512], bf16, name="pb", tag="pb")
            nc.scalar.activation(pb[:, :w], sps[:, :w], Exp, scale=SCALE,
                                 accum_out=RS[t][:, blk:blk + 1])
            ptile = pt_pool.tile([P, 512], bf16, name="ptd", tag="ptd")
            nc.sync.dma_start_transpose(
                out=ptile.rearrange("p (st c) -> p st c", c=P)[:, :w // P, :],
                in_=pb[:, :w])
            pend_q.append((ptile, w, blk, t))
            if len(pend_q) > 3:
                emit_pv(pend_q.popleft())

    # drain: flush the remaining work items and Z->out stages, interleaving
    # the held stripes' Z W_v^T matmuls so the final stripe's Z-transpose
    # flight has PE work to hide behind; only the last store chain is
    # exposed.
    while pend_q:
        emit_pv(pend_q.popleft())
    tick_zout()
    while len(zout_q) > 1:
        ent = zout_q.pop(0)
        emit_zout(ent[1], ent[2])
    zt1, rinv1 = held[1]
    emit_zout(1, zt1, rinv1)
    while zout_q:
        ent = zout_q.pop(0)
        emit_zout(ent[1], ent[2])
    zt0, rinv0 = held[0]
    emit_zout(0, zt0, rinv0)


def build_program():
    if "nc" in _prog_cache:
        return _prog_cache["nc"]
    from contextlib import ExitStack
    from concourse import bacc, mybir
    import concourse.tile as tile

    nc = bacc.Bacc("TRN2", target_bir_lowering=False, debug=False,
                   num_devices=NCORES)
    f32 = mybir.dt.float32
    bf16 = mybir.dt.bfloat16
    ap = {
        "xT": nc.dram_tensor("xT", [E, S], bf16, kind="ExternalInput").ap(),
        "xP": nc.dram_tensor("xP", [S, E], bf16, kind="ExternalInput").ap(),
        "xTq": nc.dram_tensor("xTq", [E, 1024], bf16, kind="ExternalInput").ap(),
        "wqS": nc.dram_tensor("wqS", [ET, P, KT, P], bf16,
                              kind="ExternalInput").ap(),
        "wkP": nc.dram_tensor("wkP", [KD, E], bf16, kind="ExternalInput").ap(),
        "wvT": nc.dram_tensor("wvT", [E, E], bf16, kind="ExternalInput").ap(),
        "cmask": nc.dram_tensor("cmask", [P, 256], f32, kind="ExternalInput").ap(),
        "out": nc.dram_tensor("out", [1024, E], bf16, kind="ExternalOutput").ap(),
    }
    with tile.TileContext(nc) as tc:
        with ExitStack() as ctx:
            _build_body(ctx, tc, ap)
    nc.compile()
    _prog_cache["nc"] = nc
    return nc


def make_in_maps(x, W_q, W_k, W_v):
    import ml_dtypes
    bf16 = ml_dtypes.bfloat16
    x = np.asarray(x, np.float32)
    # wqS[e, p, k, c] = W_q[k*128+p, e*128+c]
    wqS = np.ascontiguousarray(
        np.asarray(W_q, np.float32).astype(bf16)
        .reshape(KT, P, ET, P).transpose(2, 1, 0, 3))
    wkP = np.ascontiguousarray(np.asarray(W_k, np.float32).astype(bf16))
    wvT = np.ascontiguousarray(np.asarray(W_v, np.float32).T.astype(bf16))
    i = np.arange(P)[:, None]
    j = np.arange(256)[None, :]
    cmasks = [np.where(j <= i + 128, 0.0, NEG).astype(np.float32),
              np.where(j <= i, 0.0, NEG).astype(np.float32)]
    in_maps = []
    for c in range(NCORES):
        b, h = c // 2, c % 2
        xb = x[b].astype(bf16)
        xT = xb.T
        qtiles = [2 * t + (1 - h) for t in range(NQT)]
        qcols = np.concatenate([np.arange(g * P, (g + 1) * P) for g in qtiles])
        xTq = np.ascontiguousarray(xT[:, qcols])
        in_maps.append({
            "xT": np.ascontiguousarray(xT), "xP": np.ascontiguousarray(xb),
            "xTq": xTq, "wqS": wqS, "wkP": wkP, "wvT": wvT,
            "cmask": cmasks[h],
        })
    return in_maps


def assemble(results):
    out = np.zeros((B, S, E), np.float32)
    for c in range(NCORES):
        b, h = c // 2, c % 2
        co = np.asarray(results[c]["out"], dtype=np.float32)
        for t in range(NQT):
            g = 2 * t + (1 - h)
            out[b, g * P:(g + 1) * P, :] = co[t * P:(t + 1) * P]
    return out


def kernel(x, W_q, W_k, W_v):
    from concourse.bass_utils import run_bass_kernel_spmd
    nc = build_program()
    in_maps = make_in_maps(x, W_q, W_k, W_v)
    res = run_bass_kernel_spmd(nc, in_maps, core_ids=list(range(NCORES)))
    return assemble(res.results)


# revision 88
# speedup vs baseline: 1.3387x; 1.0131x over previous
"""Collective-free causal attention: 8 cores = 4 batches x 2 q-stripe sets.

Core c = (batch b = c//2, stripe set h = c%2) owns the 8 query stripes
g = 2t + (1-h), t in 0..7, of batch b.  No collectives are used — the cost
model charges intra-pair AllGathers like inter-chip transfers (15us +
size/40GBps, serialized on one resource), which dominated the original
version.

The key restructure: the full-sequence K and V projections (the work that
would otherwise be duplicated within each core pair) are NEVER materialized.
With M = W_q^T W_k precomputed per core (E x E, cheap), scores are
S = (x_q M) x^T — the per-core q-side Y^T = (x_q M)^T contracts directly
against resident x^T key blocks.  Symmetrically PV = (P x) W_v^T: the
attention accumulates Z = P x against resident plain-x key tiles and
multiplies by W_v^T once per 128-row stripe.  Per-core PE work drops from
~198us (with duplicated K/V projections) to ~147us.

Numerics: all matmul inputs are bf16 (host-converted); accumulation stays
fp32 in PSUM, softmax row sums stay fp32; outputs are stored bf16 and
widened to fp32 on the host.  Measured rel. Frobenius error ~5.3e-3.

Overlap structure (single PE-dense stream):
 - ~56 throwaway warm-up matmuls on a memset tile fill the input-DMA head
   so the cost model's PE clock ramp (half speed for the first 3us of a
   busy stretch) is spent before real work arrives.
 - Inputs land as a few large batched transfers ordered so M's first PSUM
   group is runnable ~8.7us in and every later phase's operands land ahead
   of consumption.  Phases: M -> Y^T -> held scores (t=0,1) -> attention.
 - Attention is stripe-major: each stripe walks its causal key blocks
   back-to-back, accumulating Z in one PSUM accumulation group.  P^T is
   produced by an async SBUF->SBUF XBAR DMA transpose issued right after
   the exp, three items ahead of its Z matmuls — the PE never transposes.
   The stripe-final Z is evicted to bf16, XBAR-transposed (from the ACT
   DGE queue to dodge SP sequencer head-of-line blocking), multiplied by
   W_v^T three items later, scaled by 1/rowsum on ACT and DVE in parallel
   straight out of PSUM, and stored with a single DMA.
 - The single-block stripes t=0,1 precompute everything through Z^T early;
   the kernel tail is just their 16 Z W_v^T matmuls -> scale -> store.
"""

import numpy as np

B, S, E, KD = 4, 2048, 1024, 1024
NCORES = 8
P = 128
ET = E // P          # 8 e-tiles of the contraction dim
KT = KD // P         # 8 k-tiles of Q^T/K^T partition dim
NQT = 8              # 8 q stripes of 128 per core
NBLK = 4             # 4 key blocks of 512
NST = S // P         # 16 key subtiles of 128 (V tiles)
NEG = -30000.0
SCALE = 1.0 / float(np.sqrt(KD))

_prog_cache = {}


def _n_blocks(t):
    return (t + 2) // 2


def _build_body(ctx, tc, ap):
    from concourse import mybir

    nc = tc.nc
    f32 = mybir.dt.float32
    bf16 = mybir.dt.bfloat16
    Exp = mybir.ActivationFunctionType.Exp
    X = mybir.AxisListType.X

    # batched [partition, e, cols] views of the inputs
    xTb = ap["xT"].rearrange("(e p) s -> p e s", p=P)      # [128, 8, 2048]
    xqb = ap["xTq"].rearrange("(e p) q -> p e q", p=P)     # [128, 8, 1024]
    wvb = ap["wvT"].rearrange("(e p) f -> p e f", p=P)
    out_t = ap["out"].rearrange("(t p) f -> t p f", p=P)

    # ---- persistent tiles
    # YT[f-tile] = (x_q M)^T — plays the role Q^T played before: scores are
    # S = x_q M x^T with M = W_q^T W_k, so the full-sequence K projection
    # (the expensive duplicated half) is never materialized; scores contract
    # YT directly against the resident x^T key blocks.  Symmetrically the
    # full-sequence V projection is never materialized either: PV = (P x)
    # W_v^T, so the attention accumulates Z = P x against the resident
    # plain-x key tiles and multiplies by W_v^T once per 128-row stripe.
    qt_pool = ctx.enter_context(tc.tile_pool(name="qt", bufs=1))
    YT = [qt_pool.tile([P, 1024], bf16, name=f"yt{k}", tag=f"yt{k}") for k in range(KT)]
    # x^T per key block sb: [p, (e 512)] (slice e: [:, e*512:(e+1)*512])
    xt_pool = ctx.enter_context(tc.tile_pool(name="xtp", bufs=1))
    xts = [xt_pool.tile([P, ET * 512], bf16, name=f"xts{sb}", tag=f"xts{sb}")
           for sb in range(NBLK)]
    # plain x per key subtile: [128 keys, 1024 e] (Z rhs)
    xs_pool = ctx.enter_context(tc.tile_pool(name="xsp", bufs=1))
    XS = [xs_pool.tile([P, E], bf16, name=f"xs{s}", tag=f"xs{s}") for s in range(NST)]
    # W_v^T resident: [p, (e f)] (Z W_v^T rhs)
    wvp_pool = ctx.enter_context(tc.tile_pool(name="wvp", bufs=1))
    wv_all = wvp_pool.tile([P, ET * E], bf16, name="wv", tag="wv")
    rs_pool = ctx.enter_context(tc.tile_pool(name="rsp", bufs=1))
    RS = [rs_pool.tile([P, NBLK], f32, name=f"rs{t}", tag=f"rs{t}") for t in range(NQT)]
    const_pool = ctx.enter_context(tc.tile_pool(name="const", bufs=1))
    fin_pool = ctx.enter_context(tc.tile_pool(name="fin", bufs=4))

    # PSUM plan: sp lives for the whole kernel; pp (projection evictions,
    # 2 banks) is scoped to the projection phase and its banks are reused
    # by the attention vp pool (2 tags x 3 bufs = 6 banks; the handoff
    # dependency lands on the first PV matmuls, long after the last
    # projection eviction — no stall).
    sp = ctx.enter_context(tc.tile_pool(name="sp", bufs=2, space="PSUM"))

    # PE warm-up: the cost model runs the PE at 1/3.7 speed for the first
    # ~100ns of a busy stretch and at half speed until 3us of continuous
    # activity.  Fill the input-DMA head (~7us) with throwaway matmuls on a
    # memset tile so every real matmul runs at full rate.
    warm_sb = const_pool.tile([P, 256], bf16, name="warm_sb")
    nc.gpsimd.memset(warm_sb, 0)
    for i in range(37):
        wps = sp.tile([P, 256], f32, name="wps", tag="sp")
        nc.tensor.matmul(wps, warm_sb[:, :P], warm_sb, start=True, stop=True)

    p_pool = ctx.enter_context(tc.tile_pool(name="ppb", bufs=4))

    # GPSIMD cannot access PSUM, so evictions alternate DVE/ACT.
    evict_ops = [lambda d, s: nc.vector.tensor_copy(d, s),
                 lambda d, s: nc.scalar.copy(d, s)]
    evict_i = 0

    def evict(dst, src):
        nonlocal evict_i
        evict_ops[evict_i % 2](dst, src)
        evict_i += 1

    # ---- projection phase (scoped input pools + scoped eviction PSUM pool)
    held = {}  # t -> (P^T tiles, 1/rowsum) for the split stripes t=0,1
    with tc.tile_pool(name="xqp", bufs=1) as xq_pool, \
         tc.tile_pool(name="wqp", bufs=1) as wq_pool, \
         tc.tile_pool(name="wkp", bufs=1) as wk_pool, \
         tc.tile_pool(name="mmp", bufs=1) as m_pool, \
         tc.tile_pool(name="pp", bufs=2, space="PSUM") as pp:
        # W_q host-shuffled to [e-chunk, p, k-tile, c]: each e-chunk DMA
        # moves 2KB-contiguous rows and feeds the M e-loop just in time
        wqc = [wq_pool.tile([P, KT * P], bf16, name=f"wqc{e}", tag=f"wqc{e}")
               for e in range(ET)]
        wk_all = wk_pool.tile([P, KT * E], bf16, name="wk", tag="wk")
        MM = [m_pool.tile([P, E], bf16, name=f"mm{e}", tag=f"mm{e}")
              for e in range(ET)]
        xq_all = xq_pool.tile([P, ET * 1024], bf16, name="xq", tag="xq")

        # DMA order: the M computation's first PSUM group needs the e0
        # chunk of W_q plus the fb0 half of W_k (runnable ~6.5us in); the
        # remaining W_q chunks and the W_k fb1 half land just ahead of the
        # PE's M loop consuming them.
        wkv = wk_all.rearrange("p (k f) -> p k f", f=E)
        wqS = ap["wqS"].rearrange("e p k c -> e p (k c)")   # [8, 128, 1024]
        wkb2 = ap["wkP"].rearrange("(k p) f -> p k f", p=P)
        nc.sync.dma_start(out=wqc[0], in_=wqS[0])
        nc.sync.dma_start(out=wkv[:, :, 0:512], in_=wkb2[:, :, 0:512])
        for e in range(1, ET):
            nc.sync.dma_start(out=wqc[e], in_=wqS[e])
        nc.sync.dma_start(out=wkv[:, :, 512:1024], in_=wkb2[:, :, 512:1024])
        nc.sync.dma_start(
            out=xq_all.rearrange("p (e q) -> p e q", q=1024), in_=xqb)
        for sb in range(NBLK):
            nc.sync.dma_start(
                out=xts[sb].rearrange("p (e s) -> p e s", s=512),
                in_=xTb[:, :, sb * 512:(sb + 1) * 512])
        xsb = ap["xP"].rearrange("(g p) e -> g p e", p=P)   # [16,128,1024]
        for g in range(NST):
            nc.sync.dma_start(out=XS[g], in_=xsb[g])
        nc.sync.dma_start(
            out=wv_all.rearrange("p (e f) -> p e f", f=E), in_=wvb)
        cm = const_pool.tile([P, 256], f32, name="cm")
        nc.sync.dma_start(out=cm, in_=ap["cmask"])

        # M[e,f] = sum_k W_q[k,e] W_k[k,f]  (the merged score operator);
        # fb-outer so the second W_k half may land while fb0 groups run
        for fb in range(2):
            for et in range(ET):
                ps = pp.tile([P, 512], f32, name="ps_m", tag="pp")
                for kt in range(KT):
                    nc.tensor.matmul(
                        ps, wqc[et][:, kt * P:(kt + 1) * P],
                        wk_all[:, kt * E + fb * 512: kt * E + (fb + 1) * 512],
                        start=(kt == 0), stop=(kt == KT - 1))
                evict(MM[et][:, fb * 512:(fb + 1) * 512], ps)

        # YT[f-tile] = (x_q M)^T : [128 f, 1024 own q rows]
        for qb in range(2):
            for ft in range(KT):
                ps = pp.tile([P, 512], f32, name="ps_y", tag="pp")
                for et in range(ET):
                    nc.tensor.matmul(
                        ps, MM[et][:, ft * P:(ft + 1) * P],
                        xq_all[:, et * 1024 + qb * 512: et * 1024 + (qb + 1) * 512],
                        start=(et == 0), stop=(et == ET - 1))
                evict(YT[ft][:, qb * 512:(qb + 1) * 512], ps)

        # scores/exp/transpose for the single-block stripes t=0,1 run here
        # (their inputs — YT and x^T key block 0 — are ready); P^T and
        # 1/rowsum are held in SBUF and only their PV + store run at the
        # very end of the kernel.
        for t in (1, 0):
            w = 256 * (t + 1)
            sps = sp.tile([P, 512], f32, name="sps", tag="sp")
            for k in range(KT):
                nc.tensor.matmul(sps[:, :w], YT[k][:, t * P:(t + 1) * P],
                                 xts[0][:, k * 512: k * 512 + w],
                                 start=(k == 0), stop=(k == KT - 1))
            nc.vector.tensor_add(sps[:, w - 256:w], sps[:, w - 256:w], cm)
            pb = p_pool.tile([P, 512], bf16, name="pb", tag="pb")
            nc.scalar.activation(pb[:, :w], sps[:, :w], Exp, scale=SCALE,
                                 accum_out=RS[t][:, 0:1])
            ptile = fin_pool.tile([P, w], bf16, name="hpt", tag=f"hpt{t}",
                                  bufs=1)
            nc.sync.dma_start_transpose(
                out=ptile.rearrange("p (st c) -> p st c", c=P),
                in_=pb[:, :w])
            rinv = fin_pool.tile([P, 1], f32, name="hri", tag=f"hri{t}",
                                 bufs=1)
            nc.vector.reciprocal(rinv, RS[t][:, 0:1])
            held[t] = (ptile, w // P, rinv)

    # ---- attention phase
    vp = ctx.enter_context(tc.tile_pool(name="vp", bufs=1, space="PSUM"))
    op = ctx.enter_context(tc.tile_pool(name="op", bufs=2, space="PSUM"))
    pt_pool = ctx.enter_context(tc.tile_pool(name="ptp", bufs=5))

    # Attention is stripe-major: each stripe walks its causal key blocks
    # back-to-back, accumulating Z = P x in one PSUM accumulation group
    # (zps holds the two e-halves).  At the stripe's end Z is evicted to
    # bf16, transposed by an async XBAR DMA, multiplied by W_v^T (16
    # matmuls into the op PSUM pool) and scaled straight out of PSUM by
    # 1/rowsum.  The Z->out stage runs two work items behind its stripe so
    # the eviction + DMA-transpose flight hides behind the scores stream.
    cur_zps = {}
    fin_parity = [0]

    def scale_out(t, srcs, rinv):
        obf = fin_pool.tile([P, E], bf16, name="obf", tag="obf", bufs=4)
        # scale the two halves on different engines in parallel (alternating
        # the assignment between consecutive finalizes so back-to-back
        # stripe finishes don't queue on one engine), then store with a
        # single DMA (HWDGE overhead is per-DMA)
        halves = [(0, srcs[0]), (1, srcs[1])]
        if fin_parity[0]:
            halves.reverse()
        fin_parity[0] ^= 1
        for i, (fb, src) in enumerate(halves):
            dst = obf[:, fb * 512:(fb + 1) * 512]
            if i == 0:
                nc.scalar.activation(dst, src,
                                     mybir.ActivationFunctionType.Copy,
                                     scale=rinv)
            else:
                nc.vector.tensor_scalar_mul(dst, src, rinv)
        nc.sync.dma_start(out=out_t[t], in_=obf)

    def evict_z(t, zps, zt_tag, zt_bufs):
        # Z (PSUM) -> bf16 -> async XBAR transpose -> zt [e-in-tile,(et q)]
        zb = fin_pool.tile([P, E], bf16, name="zb", tag="zb", bufs=3)
        nc.vector.tensor_copy(zb[:, 0:512], zps[0])
        nc.scalar.copy(zb[:, 512:1024], zps[1])
        zt = fin_pool.tile([P, E], bf16, name="zt", tag=zt_tag, bufs=zt_bufs)
        # issued from the ACT engine DGE queue: the SP sequencer is busy with
        # P^T transposes and output stores (~590ns each, in-order), which
        # would head-of-line-block this latency-critical transpose
        nc.scalar.dma_start_transpose(
            out=zt.rearrange("p (et q) -> p et q", q=P), in_=zb)
        return zt

    def emit_zout(t, zt, rinv=None):
        # out = Z W_v^T accumulated in the op PSUM pool, scaled by 1/rowsum
        ops = [op.tile([P, 512], f32, name=f"ops{fb}", tag=f"op{fb}")
               for fb in range(2)]
        for et in range(ET):
            for fb in range(2):
                nc.tensor.matmul(ops[fb], zt[:, et * P:(et + 1) * P],
                                 wv_all[:, et * E + fb * 512: et * E + (fb + 1) * 512],
                                 start=(et == 0), stop=(et == ET - 1))
        if rinv is None:
            rsum = fin_pool.tile([P, 1], f32, name="rsum", tag="rsum")
            nc.vector.reduce_sum(rsum, RS[t][:, :_n_blocks(t)], axis=X)
            rinv = fin_pool.tile([P, 1], f32, name="rinv", tag="rinv")
            nc.vector.reciprocal(rinv, rsum)
        scale_out(t, ops, rinv)

    zout_q = []  # [(age, t, zt)]

    def tick_zout():
        for ent in list(zout_q):
            if ent[0] >= 3:
                zout_q.remove(ent)
                emit_zout(ent[1], ent[2])
            else:
                ent[0] += 1

    def emit_pv(pend):
        # deferred Z accumulation for one (t, blk) work item; P^T arrives
        # via an async DMA transpose issued right after the exp, three
        # positions ahead, so its flight time hides behind scores work.
        tick_zout()
        ptile, w, blk, t = pend
        nst = w // P
        if blk == 0:
            cur_zps[t] = [vp.tile([P, 512], f32, name=f"zps{fb}",
                                  tag=f"vp{fb}") for fb in range(2)]
        zps = cur_zps[t]
        is_final = (blk == _n_blocks(t) - 1)
        for st in range(nst):
            for eb in range(2):
                nc.tensor.matmul(zps[eb], ptile[:, st * P:(st + 1) * P],
                                 XS[4 * blk + st][:, eb * 512:(eb + 1) * 512],
                                 start=(blk == 0 and st == 0),
                                 stop=is_final and (st == nst - 1))
        if is_final:
            zt = evict_z(t, zps, "ztm", 2)
            zout_q.append([0, t, zt])

    # held stripes t=0,1: their Z is also fully precomputed here (P^T and
    # 1/rowsum were staged during the projection phase); only the 16
    # Z W_v^T matmuls + scale + store remain for the kernel tail.
    for t in (1, 0):
        ptile, nst, rinv = held[t]
        zps = [vp.tile([P, 512], f32, name=f"zps{fb}", tag=f"vp{fb}")
               for fb in range(2)]
        for st in range(nst):
            for eb in range(2):
                nc.tensor.matmul(zps[eb], ptile[:, st * P:(st + 1) * P],
                                 XS[st][:, eb * 512:(eb + 1) * 512],
                                 start=(st == 0), stop=(st == nst - 1))
        zt = evict_z(t, zps, f"hzt{t}", 1)
        held[t] = (zt, rinv)

    # stripe-major schedule: big stripes first; the held single-block
    # stripes t=1,0 come last, so the kernel tail is just the Z W_v^T
    # matmuls -> scale -> store.
    from collections import deque
    pend_q = deque()
    for t in (7, 3, 6, 2, 5, 4):
        for blk in range(_n_blocks(t)):
            w = min(512, 256 * (t + 1) - 512 * blk)
            is_diag = (blk == _n_blocks(t) - 1)
            sps = sp.tile([P, 512], f32, name="sps", tag="sp")
            for k in range(KT):
                nc.tensor.matmul(sps[:, :w], YT[k][:, t * P:(t + 1) * P],
                                 xts[blk][:, k * 512: k * 512 + w],
                                 start=(k == 0), stop=(k == KT - 1))
            if is_diag:
                nc.vector.tensor_add(sps[:, w - 256:w], sps[:, w - 256:w], cm)
            pb = p_pool.tile([P, 512], bf16, name="pb", tag="pb")
            nc.scalar.activation(pb[:, :w], sps[:, :w], Exp, scale=SCALE,
                                 accum_out=RS[t][:, blk:blk + 1])
            ptile = pt_pool.tile([P, 512], bf16, name="ptd", tag="ptd")
            nc.sync.dma_start_transpose(
                out=ptile.rearrange("p (st c) -> p st c", c=P)[:, :w // P, :],
                in_=pb[:, :w])
            pend_q.append((ptile, w, blk, t))
            if len(pend_q) > 3:
                emit_pv(pend_q.popleft())

    # drain: flush the remaining work items and Z->out stages, interleaving
    # the held stripes' Z W_v^T matmuls so the final stripe's Z-transpose
    # flight has PE work to hide behind; only the last store chain is
    # exposed.
    while pend_q:
        emit_pv(pend_q.popleft())
    tick_zout()
    while len(zout_q) > 1:
        ent = zout_q.pop(0)
        emit_zout(ent[1], ent[2])
    zt1, rinv1 = held[1]
    emit_zout(1, zt1, rinv1)
    while zout_q:
        ent = zout_q.pop(0)
        emit_zout(ent[1], ent[2])
    zt0, rinv0 = held[0]
    emit_zout(0, zt0, rinv0)


def build_program():
    if "nc" in _prog_cache:
        return _prog_cache["nc"]
    from contextlib import ExitStack
    from concourse import bacc, mybir
    import concourse.tile as tile

    nc = bacc.Bacc("TRN2", target_bir_lowering=False, debug=False,
                   num_devices=NCORES)
    f32 = mybir.dt.float32
    bf16 = mybir.dt.bfloat16
    ap = {
        "xT": nc.dram_tensor("xT", [E, S], bf16, kind="ExternalInput").ap(),
        "xP": nc.dram_tensor("xP", [S, E], bf16, kind="ExternalInput").ap(),
        "xTq": nc.dram_tensor("xTq", [E, 1024], bf16, kind="ExternalInput").ap(),
        "wqS": nc.dram_tensor("wqS", [ET, P, KT, P], bf16,
                              kind="ExternalInput").ap(),
        "wkP": nc.dram_tensor("wkP", [KD, E], bf16, kind="ExternalInput").ap(),
        "wvT": nc.dram_tensor("wvT", [E, E], bf16, kind="ExternalInput").ap(),
        "cmask": nc.dram_tensor("cmask", [P, 256], f32, kind="ExternalInput").ap(),
        "out": nc.dram_tensor("out", [1024, E], bf16, kind="ExternalOutput").ap(),
    }
    with tile.TileContext(nc) as tc:
        with ExitStack() as ctx:
            _build_body(ctx, tc, ap)
    nc.compile()
    _prog_cache["nc"] = nc
    return nc


def make_in_maps(x, W_q, W_k, W_v):
    import ml_dtypes
    bf16 = ml_dtypes.bfloat16
    x = np.asarray(x, np.float32)
    # wqS[e, p, k, c] = W_q[k*128+p, e*128+c]
    wqS = np.ascontiguousarray(
        np.asarray(W_q, np.float32).astype(bf16)
        .reshape(KT, P, ET, P).transpose(2, 1, 0, 3))
    wkP = np.ascontiguousarray(np.asarray(W_k, np.float32).astype(bf16))
    wvT = np.ascontiguousarray(np.asarray(W_v, np.float32).T.astype(bf16))
    i = np.arange(P)[:, None]
    j = np.arange(256)[None, :]
    cmasks = [np.where(j <= i + 128, 0.0, NEG).astype(np.float32),
              np.where(j <= i, 0.0, NEG).astype(np.float32)]
    in_maps = []
    for c in range(NCORES):
        b, h = c // 2, c % 2
        xb = x[b].astype(bf16)
        xT = xb.T
        qtiles = [2 * t + (1 - h) for t in range(NQT)]
        qcols = np.concatenate([np.arange(g * P, (g + 1) * P) for g in qtiles])
        xTq = np.ascontiguousarray(xT[:, qcols])
        in_maps.append({
            "xT": np.ascontiguousarray(xT), "xP": np.ascontiguousarray(xb),
            "xTq": xTq, "wqS": wqS, "wkP": wkP, "wvT": wvT,
            "cmask": cmasks[h],
        })
    return in_maps


def assemble(results):
    out = np.zeros((B, S, E), np.float32)
    for c in range(NCORES):
        b, h = c // 2, c % 2
        co = np.asarray(results[c]["out"], dtype=np.float32)
        for t in range(NQT):
            g = 2 * t + (1 - h)
            out[b, g * P:(g + 1) * P, :] = co[t * P:(t + 1) * P]
    return out


def kernel(x, W_q, W_k, W_v):
    from concourse.bass_utils import run_bass_kernel_spmd
    nc = build_program()
    in_maps = make_in_maps(x, W_q, W_k, W_v)
    res = run_bass_kernel_spmd(nc, in_maps, core_ids=list(range(NCORES)))
    return assemble(res.results)


# revision 90
# speedup vs baseline: 1.3465x; 1.0059x over previous
"""Collective-free causal attention: 8 cores = 4 batches x 2 q-stripe sets.

Core c = (batch b = c//2, stripe set h = c%2) owns the 8 query stripes
g = 2t + (1-h), t in 0..7, of batch b.  No collectives are used — the cost
model charges intra-pair AllGathers like inter-chip transfers (15us +
size/40GBps, serialized on one resource), which dominated the original
version.

The key restructure: the full-sequence K and V projections (the work that
would otherwise be duplicated within each core pair) are NEVER materialized.
With M = W_q^T W_k precomputed per core (E x E, cheap), scores are
S = (x_q M) x^T — the per-core q-side Y^T = (x_q M)^T contracts directly
against resident x^T key blocks.  Symmetrically PV = (P x) W_v^T: the
attention accumulates Z = P x against resident plain-x key tiles and
multiplies by W_v^T once per 128-row stripe.  Per-core PE work drops from
~198us (with duplicated K/V projections) to ~147us.

Numerics: all matmul inputs are bf16 (host-converted); accumulation stays
fp32 in PSUM, softmax row sums stay fp32; outputs are stored bf16 and
widened to fp32 on the host.  Measured rel. Frobenius error ~5.3e-3.

Overlap structure (single PE-dense stream):
 - ~56 throwaway warm-up matmuls on a memset tile fill the input-DMA head
   so the cost model's PE clock ramp (half speed for the first 3us of a
   busy stretch) is spent before real work arrives.
 - Inputs land as a few large batched transfers ordered so M's first PSUM
   group is runnable ~8.7us in and every later phase's operands land ahead
   of consumption.  Phases: M -> Y^T -> held scores (t=0,1) -> attention.
 - Attention is stripe-major: each stripe walks its causal key blocks
   back-to-back, accumulating Z in one PSUM accumulation group.  P^T is
   produced by an async SBUF->SBUF XBAR DMA transpose issued right after
   the exp, three items ahead of its Z matmuls — the PE never transposes.
   The stripe-final Z is evicted to bf16, XBAR-transposed (from the ACT
   DGE queue to dodge SP sequencer head-of-line blocking), multiplied by
   W_v^T three items later, scaled by 1/rowsum on ACT and DVE in parallel
   straight out of PSUM, and stored with a single DMA.
 - The single-block stripes t=0,1 precompute everything through Z^T early;
   the kernel tail is just their 16 Z W_v^T matmuls -> scale -> store.
"""

import numpy as np

B, S, E, KD = 4, 2048, 1024, 1024
NCORES = 8
P = 128
ET = E // P          # 8 e-tiles of the contraction dim
KT = KD // P         # 8 k-tiles of Q^T/K^T partition dim
NQT = 8              # 8 q stripes of 128 per core
NBLK = 4             # 4 key blocks of 512
NST = S // P         # 16 key subtiles of 128 (V tiles)
NEG = -30000.0
SCALE = 1.0 / float(np.sqrt(KD))

_prog_cache = {}


def _n_blocks(t):
    return (t + 2) // 2


def _build_body(ctx, tc, ap):
    from concourse import mybir

    nc = tc.nc
    f32 = mybir.dt.float32
    bf16 = mybir.dt.bfloat16
    Exp = mybir.ActivationFunctionType.Exp
    X = mybir.AxisListType.X

    # batched [partition, e, cols] views of the inputs
    xTb = ap["xT"].rearrange("(e p) s -> p e s", p=P)      # [128, 8, 2048]
    xqb = ap["xTq"].rearrange("(e p) q -> p e q", p=P)     # [128, 8, 1024]
    wvb = ap["wvT"].rearrange("(e p) f -> p e f", p=P)
    out_t = ap["out"].rearrange("(t p) f -> t p f", p=P)

    # ---- persistent tiles
    # YT[f-tile] = (x_q M)^T — plays the role Q^T played before: scores are
    # S = x_q M x^T with M = W_q^T W_k, so the full-sequence K projection
    # (the expensive duplicated half) is never materialized; scores contract
    # YT directly against the resident x^T key blocks.  Symmetrically the
    # full-sequence V projection is never materialized either: PV = (P x)
    # W_v^T, so the attention accumulates Z = P x against the resident
    # plain-x key tiles and multiplies by W_v^T once per 128-row stripe.
    qt_pool = ctx.enter_context(tc.tile_pool(name="qt", bufs=1))
    YT = [qt_pool.tile([P, 1024], bf16, name=f"yt{k}", tag=f"yt{k}") for k in range(KT)]
    # x^T per key block sb: [p, (e 512)] (slice e: [:, e*512:(e+1)*512])
    xt_pool = ctx.enter_context(tc.tile_pool(name="xtp", bufs=1))
    xts = [xt_pool.tile([P, ET * 512], bf16, name=f"xts{sb}", tag=f"xts{sb}")
           for sb in range(NBLK)]
    # plain x per key subtile: [128 keys, 1024 e] (Z rhs)
    xs_pool = ctx.enter_context(tc.tile_pool(name="xsp", bufs=1))
    XS = [xs_pool.tile([P, E], bf16, name=f"xs{s}", tag=f"xs{s}") for s in range(NST)]
    # W_v^T resident: [p, (e f)] (Z W_v^T rhs)
    wvp_pool = ctx.enter_context(tc.tile_pool(name="wvp", bufs=1))
    wv_all = wvp_pool.tile([P, ET * E], bf16, name="wv", tag="wv")
    rs_pool = ctx.enter_context(tc.tile_pool(name="rsp", bufs=1))
    RS = [rs_pool.tile([P, NBLK], f32, name=f"rs{t}", tag=f"rs{t}") for t in range(NQT)]
    const_pool = ctx.enter_context(tc.tile_pool(name="const", bufs=1))
    fin_pool = ctx.enter_context(tc.tile_pool(name="fin", bufs=4))

    # PSUM plan: sp lives for the whole kernel; pp (projection evictions,
    # 2 banks) is scoped to the projection phase and its banks are reused
    # by the attention vp pool (2 tags x 3 bufs = 6 banks; the handoff
    # dependency lands on the first PV matmuls, long after the last
    # projection eviction — no stall).
    sp = ctx.enter_context(tc.tile_pool(name="sp", bufs=2, space="PSUM"))

    # PE warm-up: the cost model runs the PE at 1/3.7 speed for the first
    # ~100ns of a busy stretch and at half speed until 3us of continuous
    # activity.  Fill the input-DMA head (~7us) with throwaway matmuls on a
    # memset tile so every real matmul runs at full rate.
    warm_sb = const_pool.tile([P, 256], bf16, name="warm_sb")
    nc.gpsimd.memset(warm_sb, 0)
    for i in range(37):
        wps = sp.tile([P, 256], f32, name="wps", tag="sp")
        nc.tensor.matmul(wps, warm_sb[:, :P], warm_sb, start=True, stop=True)

    p_pool = ctx.enter_context(tc.tile_pool(name="ppb", bufs=4))

    # GPSIMD cannot access PSUM, so evictions alternate DVE/ACT.
    evict_ops = [lambda d, s: nc.vector.tensor_copy(d, s),
                 lambda d, s: nc.scalar.copy(d, s)]
    evict_i = 0

    def evict(dst, src):
        nonlocal evict_i
        evict_ops[evict_i % 2](dst, src)
        evict_i += 1

    # ---- projection phase (scoped input pools + scoped eviction PSUM pool)
    held = {}  # t -> (P^T tiles, 1/rowsum) for the split stripes t=0,1
    with tc.tile_pool(name="xqp", bufs=1) as xq_pool, \
         tc.tile_pool(name="wqp", bufs=1) as wq_pool, \
         tc.tile_pool(name="wkp", bufs=1) as wk_pool, \
         tc.tile_pool(name="mmp", bufs=1) as m_pool, \
         tc.tile_pool(name="pp", bufs=2, space="PSUM") as pp:
        # W_q host-shuffled to [e-chunk, p, k-tile, c]: each e-chunk DMA
        # moves 2KB-contiguous rows and feeds the M e-loop just in time
        wqc = [wq_pool.tile([P, KT * P], bf16, name=f"wqc{e}", tag=f"wqc{e}")
               for e in range(ET)]
        wk_all = wk_pool.tile([P, KT * E], bf16, name="wk", tag="wk")
        MM = [m_pool.tile([P, E], bf16, name=f"mm{e}", tag=f"mm{e}")
              for e in range(ET)]
        xq_all = xq_pool.tile([P, ET * 1024], bf16, name="xq", tag="xq")

        # DMA order: the M computation's first PSUM group needs the e0
        # chunk of W_q plus the fb0 half of W_k (runnable ~6.5us in); the
        # remaining W_q chunks and the W_k fb1 half land just ahead of the
        # PE's M loop consuming them.
        wkv = wk_all.rearrange("p (k f) -> p k f", f=E)
        wqS = ap["wqS"].rearrange("e p k c -> e p (k c)")   # [8, 128, 1024]
        wkb2 = ap["wkP"].rearrange("(k p) f -> p k f", p=P)
        nc.sync.dma_start(out=wqc[0], in_=wqS[0])
        nc.sync.dma_start(out=wkv[:, :, 0:512], in_=wkb2[:, :, 0:512])
        for e in range(1, ET):
            nc.sync.dma_start(out=wqc[e], in_=wqS[e])
        nc.sync.dma_start(out=wkv[:, :, 512:1024], in_=wkb2[:, :, 512:1024])
        nc.sync.dma_start(
            out=xq_all.rearrange("p (e q) -> p e q", q=1024), in_=xqb)
        for sb in range(NBLK):
            nc.sync.dma_start(
                out=xts[sb].rearrange("p (e s) -> p e s", s=512),
                in_=xTb[:, :, sb * 512:(sb + 1) * 512])
        xsb = ap["xP"].rearrange("(g p) e -> g p e", p=P)   # [16,128,1024]
        for g in range(NST):
            nc.sync.dma_start(out=XS[g], in_=xsb[g])
        nc.sync.dma_start(
            out=wv_all.rearrange("p (e f) -> p e f", f=E), in_=wvb)
        cm = const_pool.tile([P, 256], f32, name="cm")
        nc.sync.dma_start(out=cm, in_=ap["cmask"])

        # M[e,f] = sum_k W_q[k,e] W_k[k,f]  (the merged score operator);
        # fb-outer so the second W_k half may land while fb0 groups run
        for fb in range(2):
            for et in range(ET):
                ps = pp.tile([P, 512], f32, name="ps_m", tag="pp")
                for kt in range(KT):
                    nc.tensor.matmul(
                        ps, wqc[et][:, kt * P:(kt + 1) * P],
                        wk_all[:, kt * E + fb * 512: kt * E + (fb + 1) * 512],
                        start=(kt == 0), stop=(kt == KT - 1))
                evict(MM[et][:, fb * 512:(fb + 1) * 512], ps)

        # YT[f-tile] = (x_q M)^T : [128 f, 1024 own q rows]
        def yt_pass(qb):
            for ft in range(KT):
                ps = pp.tile([P, 512], f32, name="ps_y", tag="pp")
                for et in range(ET):
                    nc.tensor.matmul(
                        ps, MM[et][:, ft * P:(ft + 1) * P],
                        xq_all[:, et * 1024 + qb * 512: et * 1024 + (qb + 1) * 512],
                        start=(et == 0), stop=(et == ET - 1))
                evict(YT[ft][:, qb * 512:(qb + 1) * 512], ps)

        yt_pass(0)

        # scores/exp/transpose for the single-block stripes t=0,1 run here,
        # between the two YT half-passes (they only read YT's qb=0 columns)
        # so their exp/mask chains drain under YT qb=1's matmuls instead of
        # stalling the attention start on the sp-bank rotation; P^T and
        # 1/rowsum are held in SBUF and only their PV + store run at the
        # very end of the kernel.
        for t in (1, 0):
            w = 256 * (t + 1)
            sps = sp.tile([P, 512], f32, name="sps", tag="sp")
            for k in range(KT):
                nc.tensor.matmul(sps[:, :w], YT[k][:, t * P:(t + 1) * P],
                                 xts[0][:, k * 512: k * 512 + w],
                                 start=(k == 0), stop=(k == KT - 1))
            nc.vector.tensor_add(sps[:, w - 256:w], sps[:, w - 256:w], cm)
            pb = p_pool.tile([P, 512], bf16, name="pb", tag="pb")
            nc.scalar.activation(pb[:, :w], sps[:, :w], Exp, scale=SCALE,
                                 accum_out=RS[t][:, 0:1])
            ptile = fin_pool.tile([P, w], bf16, name="hpt", tag=f"hpt{t}",
                                  bufs=1)
            nc.sync.dma_start_transpose(
                out=ptile.rearrange("p (st c) -> p st c", c=P),
                in_=pb[:, :w])
            rinv = fin_pool.tile([P, 1], f32, name="hri", tag=f"hri{t}",
                                 bufs=1)
            nc.vector.reciprocal(rinv, RS[t][:, 0:1])
            held[t] = (ptile, w // P, rinv)

        yt_pass(1)

    # ---- attention phase
    vp = ctx.enter_context(tc.tile_pool(name="vp", bufs=1, space="PSUM"))
    op = ctx.enter_context(tc.tile_pool(name="op", bufs=2, space="PSUM"))
    pt_pool = ctx.enter_context(tc.tile_pool(name="ptp", bufs=5))

    # Attention is stripe-major: each stripe walks its causal key blocks
    # back-to-back, accumulating Z = P x in one PSUM accumulation group
    # (zps holds the two e-halves).  At the stripe's end Z is evicted to
    # bf16, transposed by an async XBAR DMA, multiplied by W_v^T (16
    # matmuls into the op PSUM pool) and scaled straight out of PSUM by
    # 1/rowsum.  The Z->out stage runs two work items behind its stripe so
    # the eviction + DMA-transpose flight hides behind the scores stream.
    cur_zps = {}
    fin_parity = [0]

    def scale_out(t, srcs, rinv):
        obf = fin_pool.tile([P, E], bf16, name="obf", tag="obf", bufs=4)
        # scale the two halves on different engines in parallel (alternating
        # the assignment between consecutive finalizes so back-to-back
        # stripe finishes don't queue on one engine), then store with a
        # single DMA (HWDGE overhead is per-DMA)
        halves = [(0, srcs[0]), (1, srcs[1])]
        if fin_parity[0]:
            halves.reverse()
        fin_parity[0] ^= 1
        for i, (fb, src) in enumerate(halves):
            dst = obf[:, fb * 512:(fb + 1) * 512]
            if i == 0:
                nc.scalar.activation(dst, src,
                                     mybir.ActivationFunctionType.Copy,
                                     scale=rinv)
            else:
                nc.vector.tensor_scalar_mul(dst, src, rinv)
        nc.sync.dma_start(out=out_t[t], in_=obf)

    def evict_z(t, zps, zt_tag, zt_bufs):
        # Z (PSUM) -> bf16 -> async XBAR transpose -> zt [e-in-tile,(et q)]
        zb = fin_pool.tile([P, E], bf16, name="zb", tag="zb", bufs=3)
        nc.vector.tensor_copy(zb[:, 0:512], zps[0])
        nc.scalar.copy(zb[:, 512:1024], zps[1])
        zt = fin_pool.tile([P, E], bf16, name="zt", tag=zt_tag, bufs=zt_bufs)
        # issued from the ACT engine DGE queue: the SP sequencer is busy with
        # P^T transposes and output stores (~590ns each, in-order), which
        # would head-of-line-block this latency-critical transpose
        nc.scalar.dma_start_transpose(
            out=zt.rearrange("p (et q) -> p et q", q=P), in_=zb)
        return zt

    def emit_zout(t, zt, rinv=None):
        # out = Z W_v^T accumulated in the op PSUM pool, scaled by 1/rowsum
        ops = [op.tile([P, 512], f32, name=f"ops{fb}", tag=f"op{fb}")
               for fb in range(2)]
        for et in range(ET):
            for fb in range(2):
                nc.tensor.matmul(ops[fb], zt[:, et * P:(et + 1) * P],
                                 wv_all[:, et * E + fb * 512: et * E + (fb + 1) * 512],
                                 start=(et == 0), stop=(et == ET - 1))
        if rinv is None:
            rsum = fin_pool.tile([P, 1], f32, name="rsum", tag="rsum")
            nc.vector.reduce_sum(rsum, RS[t][:, :_n_blocks(t)], axis=X)
            rinv = fin_pool.tile([P, 1], f32, name="rinv", tag="rinv")
            nc.vector.reciprocal(rinv, rsum)
        scale_out(t, ops, rinv)

    zout_q = []  # [(age, t, zt)]

    def tick_zout():
        for ent in list(zout_q):
            if ent[0] >= 3:
                zout_q.remove(ent)
                emit_zout(ent[1], ent[2])
            else:
                ent[0] += 1

    def emit_pv(pend):
        # deferred Z accumulation for one (t, blk) work item; P^T arrives
        # via an async DMA transpose issued right after the exp, three
        # positions ahead, so its flight time hides behind scores work.
        tick_zout()
        ptile, w, blk, t = pend
        nst = w // P
        if blk == 0:
            cur_zps[t] = [vp.tile([P, 512], f32, name=f"zps{fb}",
                                  tag=f"vp{fb}") for fb in range(2)]
        zps = cur_zps[t]
        is_final = (blk == _n_blocks(t) - 1)
        for st in range(nst):
            for eb in range(2):
                nc.tensor.matmul(zps[eb], ptile[:, st * P:(st + 1) * P],
                                 XS[4 * blk + st][:, eb * 512:(eb + 1) * 512],
                                 start=(blk == 0 and st == 0),
                                 stop=is_final and (st == nst - 1))
        if is_final:
            zt = evict_z(t, zps, "ztm", 2)
            zout_q.append([0, t, zt])

    # held stripes t=0,1: their Z is also fully precomputed here (P^T and
    # 1/rowsum were staged during the projection phase); only the 16
    # Z W_v^T matmuls + scale + store remain for the kernel tail.
    for t in (1, 0):
        ptile, nst, rinv = held[t]
        zps = [vp.tile([P, 512], f32, name=f"zps{fb}", tag=f"vp{fb}")
               for fb in range(2)]
        for st in range(nst):
            for eb in range(2):
                nc.tensor.matmul(zps[eb], ptile[:, st * P:(st + 1) * P],
                                 XS[st][:, eb * 512:(eb + 1) * 512],
                                 start=(st == 0), stop=(st == nst - 1))
        zt = evict_z(t, zps, f"hzt{t}", 1)
        held[t] = (zt, rinv)

    # stripe-major schedule: big stripes first; the held single-block
    # stripes t=1,0 come last, so the kernel tail is just the Z W_v^T
    # matmuls -> scale -> store.
    from collections import deque
    pend_q = deque()
    for t in (7, 3, 6, 2, 5, 4):
        for blk in range(_n_blocks(t)):
            w = min(512, 256 * (t + 1) - 512 * blk)
            is_diag = (blk == _n_blocks(t) - 1)
            sps = sp.tile([P, 512], f32, name="sps", tag="sp")
            for k in range(KT):
                nc.tensor.matmul(sps[:, :w], YT[k][:, t * P:(t + 1) * P],
                                 xts[blk][:, k * 512: k * 512 + w],
                                 start=(k == 0), stop=(k == KT - 1))
            if is_diag:
                nc.vector.tensor_add(sps[:, w - 256:w], sps[:, w - 256:w], cm)
            pb = p_pool.tile([P, 512], bf16, name="pb", tag="pb")
            nc.scalar.activation(pb[:, :w], sps[:, :w], Exp, scale=SCALE,
                                 accum_out=RS[t][:, blk:blk + 1])
            ptile = pt_pool.tile([P, 512], bf16, name="ptd", tag="ptd")
            nc.sync.dma_start_transpose(
                out=ptile.rearrange("p (st c) -> p st c", c=P)[:, :w // P, :],
                in_=pb[:, :w])
            pend_q.append((ptile, w, blk, t))
            if len(pend_q) > 3:
                emit_pv(pend_q.popleft())

    # drain: flush the remaining work items and Z->out stages, interleaving
    # the held stripes' Z W_v^T matmuls so the final stripe's Z-transpose
    # flight has PE work to hide behind; only the last store chain is
    # exposed.
    while pend_q:
        emit_pv(pend_q.popleft())
    tick_zout()
    while len(zout_q) > 1:
        ent = zout_q.pop(0)
        emit_zout(ent[1], ent[2])
    zt1, rinv1 = held[1]
    emit_zout(1, zt1, rinv1)
    while zout_q:
        ent = zout_q.pop(0)
        emit_zout(ent[1], ent[2])
    zt0, rinv0 = held[0]
    emit_zout(0, zt0, rinv0)


def build_program():
    if "nc" in _prog_cache:
        return _prog_cache["nc"]
    from contextlib import ExitStack
    from concourse import bacc, mybir
    import concourse.tile as tile

    nc = bacc.Bacc("TRN2", target_bir_lowering=False, debug=False,
                   num_devices=NCORES)
    f32 = mybir.dt.float32
    bf16 = mybir.dt.bfloat16
    ap = {
        "xT": nc.dram_tensor("xT", [E, S], bf16, kind="ExternalInput").ap(),
        "xP": nc.dram_tensor("xP", [S, E], bf16, kind="ExternalInput").ap(),
        "xTq": nc.dram_tensor("xTq", [E, 1024], bf16, kind="ExternalInput").ap(),
        "wqS": nc.dram_tensor("wqS", [ET, P, KT, P], bf16,
                              kind="ExternalInput").ap(),
        "wkP": nc.dram_tensor("wkP", [KD, E], bf16, kind="ExternalInput").ap(),
        "wvT": nc.dram_tensor("wvT", [E, E], bf16, kind="ExternalInput").ap(),
        "cmask": nc.dram_tensor("cmask", [P, 256], f32, kind="ExternalInput").ap(),
        "out": nc.dram_tensor("out", [1024, E], bf16, kind="ExternalOutput").ap(),
    }
    with tile.TileContext(nc) as tc:
        with ExitStack() as ctx:
            _build_body(ctx, tc, ap)
    nc.compile()
    _prog_cache["nc"] = nc
    return nc


def make_in_maps(x, W_q, W_k, W_v):
    import ml_dtypes
    bf16 = ml_dtypes.bfloat16
    x = np.asarray(x, np.float32)
    # wqS[e, p, k, c] = W_q[k*128+p, e*128+c]
    wqS = np.ascontiguousarray(
        np.asarray(W_q, np.float32).astype(bf16)
        .reshape(KT, P, ET, P).transpose(2, 1, 0, 3))
    wkP = np.ascontiguousarray(np.asarray(W_k, np.float32).astype(bf16))
    wvT = np.ascontiguousarray(np.asarray(W_v, np.float32).T.astype(bf16))
    i = np.arange(P)[:, None]
    j = np.arange(256)[None, :]
    cmasks = [np.where(j <= i + 128, 0.0, NEG).astype(np.float32),
              np.where(j <= i, 0.0, NEG).astype(np.float32)]
    in_maps = []
    for c in range(NCORES):
        b, h = c // 2, c % 2
        xb = x[b].astype(bf16)
        xT = xb.T
        qtiles = [2 * t + (1 - h) for t in range(NQT)]
        qcols = np.concatenate([np.arange(g * P, (g + 1) * P) for g in qtiles])
        xTq = np.ascontiguousarray(xT[:, qcols])
        in_maps.append({
            "xT": np.ascontiguousarray(xT), "xP": np.ascontiguousarray(xb),
            "xTq": xTq, "wqS": wqS, "wkP": wkP, "wvT": wvT,
            "cmask": cmasks[h],
        })
    return in_maps


def assemble(results):
    out = np.zeros((B, S, E), np.float32)
    for c in range(NCORES):
        b, h = c // 2, c % 2
        co = np.asarray(results[c]["out"], dtype=np.float32)
        for t in range(NQT):
            g = 2 * t + (1 - h)
            out[b, g * P:(g + 1) * P, :] = co[t * P:(t + 1) * P]
    return out


def kernel(x, W_q, W_k, W_v):
    from concourse.bass_utils import run_bass_kernel_spmd
    nc = build_program()
    in_maps = make_in_maps(x, W_q, W_k, W_v)
    res = run_bass_kernel_spmd(nc, in_maps, core_ids=list(range(NCORES)))
    return assemble(res.results)
